# revision 41
# baseline (speedup 1.0000x reference)
"""Trainium2 Bass kernel for nn_ActorNetwork (2-layer GCN + actor head).

Self-contained: hardcodes all shapes/sharding (8 NeuronCores).

Strategy (v2):
  - Shard dst nodes (= graphs) contiguously across 8 cores (10240 nodes/core).
  - Gather sources per edge with gpsimd dma_gather, round-robined across the
    4 SWDGE queues so descriptor generation runs on all 4 Q7 core pairs.
  - Host prescales x by dinv (bf16): gathered rows feed the one-hot
    aggregation matmuls directly (no per-edge coef multiply on-chip);
    dst-side dinv applied once per 128-dst tile at PSUM close.
  - Self-loops are ordinary edges in the edge list.
  - Edges sorted by (dst-tile-block, src-chunk, dst-tile); PSUM accumulates
    across all 3 src-chunks of a 10-tile block; per-tile close fuses
    GEMM1+ReLU+GEMM2+dinv scale, streaming m2s [NL,64] f32 to DRAM.
  - AllGather of m2s split in 4 chunks (2 blocks each), overlapped with the
    remaining layer-1 edge work; layer-2 gathers chunk against the 4
    AllGather output tensors (20480 rows each, int16-indexable).
  - Layer 2 computes only host-dst rows (first 13 of each 40), compacted
    h-major: slot c = h*256 + g  (3328 rows = 26 tiles per core).
  - Head: identical to baseline modulo the h-major rearrange.
"""
import sys
import hashlib

sys.path.insert(0, "/opt/trn_rl_repo")

import numpy as np
import ml_dtypes
from contextlib import ExitStack

from concourse import bass, mybir, tile, bass_utils, bacc
from concourse.masks import make_identity

F32 = mybir.dt.float32
BF16 = mybir.dt.bfloat16
F16 = mybir.dt.float16
I16 = mybir.dt.int16
I32 = mybir.dt.int32

N_CORES = 8
N = 81920
NL = N // N_CORES          # 10240 nodes per core
IN_DIM = 128
H1 = 256
H2 = 64
GRAPH = 40
NH = 13
ACT = 145
GPC = NL // GRAPH          # 256 graphs per core
SENT = 600.0
CALL_G = 16                # groups (of 128 idxs) per dma_gather call

# layer 1 dst layout: 80 tiles of 128 local nodes, blocks of 20 tiles
# (PSUM is bank-granular: one [128, 512] f32 bank holds 4 dst tiles, so a
#  20-tile block = 5 banks of open accumulators.)
T1 = NL // 128             # 80
BLK1S = [24, 24, 24, 8]    # asymmetric: small last block -> short tail
NB1 = len(BLK1S)
CH1 = 32768                # src chunk rows (int16 idx) over xs [N, 128]
NCH1 = 4                   # 3 chunks of xs + 1 per-core self-loop chunk (xsl)
CH1_RANGES = [(0, 32768), (32768, 32768), (65536, 16384)]

# layer 2 dst layout: host slots c = h*256 + g; 3328 = 26 tiles
NHOST = NH * GPC           # 3328
T2 = NHOST // 128          # 26
BLK2 = 20                  # blocks of 20,6 tiles
NB2 = (T2 + BLK2 - 1) // BLK2
# layer-2 source = 4 AllGather output tensors (one per layer-1 block)
AGCS = [128 * b for b in BLK1S]          # per-core rows per AG chunk
AGLO = [0, 3072, 6144, 9216, 10240]      # local-node chunk bounds
NCH2 = 4


# ---------------------------------------------------------------- host prep

def _mk_schedule(core, c_of, t_of, dloc256, idxl, ntiles, blkp_list, nch):
    """Common-max padded, block-major schedule shared by all cores.

    Segment granularity = (chunk, tile-PAIR): each 128-slot group maps to
    exactly one 256-dst window (one matmul per group, no tile spans).
    core/c_of/t_of/dloc256/idxl: per-edge arrays (dst-owning core, src chunk,
    dst tile, dst%256, chunk-local src index). blk in TILES (even).
    """
    assert ntiles % 2 == 0
    npair = ntiles // 2
    p_of = t_of // 2
    nblk = len(blkp_list)
    pb = np.concatenate([[0], np.cumsum(blkp_list)])
    assert pb[-1] == npair
    counts = np.zeros((N_CORES, nch, npair), np.int64)
    np.add.at(counts, (core, c_of, p_of), 1)
    Ncm = counts.max(axis=0)                    # [nch, npair]
    assert Ncm.min() > 0, "empty (chunk, pair) segment"

    seg_off = np.zeros((nch, npair), np.int64)
    runs = []                                   # (b, c, start, ngroups)
    off = 0
    for b in range(nblk):
        plo, phi = int(pb[b]), int(pb[b + 1])
        for c in range(nch):
            start = off
            for p in range(plo, phi):
                seg_off[c, p] = off
                off += int(Ncm[c, p])
            if (off - start) % 128:
                off += 128 - (off - start) % 128
            runs.append((b, c, start, (off - start) // 128))
    L = int(off)

    # groups: base pair = pair of first slot; a group whose 128 slots cross
    # the (common) segment boundary into pair+1 emits a second "straddle"
    # event (one-hot built against iota+256 at the kernel level).
    calls = []
    gpair_of_slot = np.full(L, -1, np.int64)
    for (b, c, start, ngroups) in runs:
        plo, phi = int(pb[b]), int(pb[b + 1])
        segs = [(int(seg_off[c, p]), int(Ncm[c, p]), p)
                for p in range(plo, phi)]
        gev = []
        for g in range(ngroups):
            s0 = start + 128 * g
            pg = segs[-1][2]
            for (so, n, p) in segs:
                if s0 < so + n:
                    pg = p
                    break
            gpair_of_slot[s0:s0 + 128] = pg
            evs = [[g, pg, False, False, False]]
            for (so, n, p) in segs:
                if p == pg:
                    if s0 + 128 > so + n and p + 1 < phi:
                        # straddles into pair p+1
                        so2, n2, _ = segs[p + 1 - plo]
                        assert s0 + 128 <= so2 + n2, "group spans >2 pairs"
                        evs.append([g, p + 1, False, False, True])
                    break
            gev.append(evs)
        gi = 0
        while gi < ngroups:
            n = min(CALL_G, ngroups - gi)
            evs = []
            for g in range(gi, gi + n):
                for (gg, p, f, l, st) in gev[g]:
                    evs.append([gg - gi, p, f, l, st])
            calls.append([c, start + 128 * gi, n, evs])
            gi += n

    # first/last event per PAIR for psum open/close flags
    first_seen, last_seen = {}, {}
    for ci, (c, s0, n, evs) in enumerate(calls):
        for ei, ev in enumerate(evs):
            p = ev[1]
            if p not in first_seen:
                first_seen[p] = (ci, ei)
            last_seen[p] = (ci, ei)
    for p, (ci, ei) in first_seen.items():
        calls[ci][3][ei][2] = True
    for p, (ci, ei) in last_seen.items():
        calls[ci][3][ei][3] = True
    assert len(first_seen) == npair

    idx_all = np.zeros((N_CORES, L), np.int16)
    dstv_all = np.full((N_CORES, L), SENT, np.float32)
    for r in range(N_CORES):
        m = core == r
        sc, sp = c_of[m], p_of[m]
        sd, si = dloc256[m], idxl[m]
        key = sc * npair + sp
        order = np.lexsort((key,))
        sc, sp, sd, si = sc[order], sp[order], sd[order], si[order]
        key = key[order]
        change = np.r_[True, key[1:] != key[:-1]]
        starts = np.flatnonzero(change)
        runid = np.cumsum(change) - 1
        within = np.arange(len(key)) - starts[runid]
        pos = seg_off[sc, sp] + within
        idx_all[r, pos] = si.astype(np.int16)
        # dstv relative to the slot's GROUP base pair: [0,256) for the base
        # pair, [256,512) for the next pair (straddle window)
        rel = sd + 256.0 * (sp - gpair_of_slot[pos])
        assert rel.min() >= 0 and rel.max() < 512
        dstv_all[r, pos] = rel

    idx_sb = np.stack([
        np.tile(idx_all[r].reshape(-1, 16).T, (8, 1)) for r in range(N_CORES)
    ])                                          # [8, 128, L/16]
    dstv_sb = np.stack([
        dstv_all[r].reshape(-1, 128).T for r in range(N_CORES)
    ]).astype(np.float16)                       # [8, 128, L/128] (ints exact)
    return dict(L=L, calls=calls, idx_sb=idx_sb, dstv_sb=dstv_sb)


def _prep(ei):
    src = ei[0].astype(np.int64)
    dst = ei[1].astype(np.int64)
    deg = np.bincount(dst, minlength=N).astype(np.float64) + 1.0
    dinv = (1.0 / np.sqrt(deg)).astype(np.float32)

    # ---------------- layer 1: all edges + self loops, dst-local layout
    # self loops form their own chunk (3) sourced from the per-core local
    # slice xsl, so their (chunk, pair) counts are identical on every core
    all_n = np.arange(N, dtype=np.int64)
    s1 = np.concatenate([src, all_n])
    d1 = np.concatenate([dst, all_n])
    core1 = d1 // NL
    du1 = d1 % NL
    E = len(src)
    c1_of = np.concatenate([src // CH1, np.full(N, 3, np.int64)])
    idxl1 = np.concatenate([src % CH1, all_n % NL])
    sch1 = _mk_schedule(core1, c1_of, du1 // 128,
                        (du1 % 256).astype(np.float32), idxl1,
                        T1, [b // 2 for b in BLK1S], NCH1)

    # ---------------- layer 2: host-dst edges + host self loops
    hm = (dst % GRAPH) < NH
    s2r, d2r = src[hm], dst[hm]
    hosts = all_n[(all_n % GRAPH) < NH]
    s2 = np.concatenate([s2r, hosts])
    d2 = np.concatenate([d2r, hosts])
    core2 = d2 // NL
    nloc = d2 % NL
    g2 = nloc // GRAPH
    h2 = nloc % GRAPH
    c2 = h2 * GPC + g2                          # compacted host slot
    # m2sf position: src s = r*NL + n -> AG tensor a = n//AGC,
    # row = (s//NL)*AGC + n%AGC
    sn = s2 % NL
    a2 = np.searchsorted(AGLO, sn, side="right") - 1
    a2 = np.clip(a2, 0, 3)
    agcs = np.array(AGCS)[a2]
    aglo = np.array(AGLO)[a2]
    pos2 = (s2 // NL) * agcs + (sn - aglo)
    sch2 = _mk_schedule(core2, a2, c2 // 128,
                        (c2 % 256).astype(np.float32), pos2,
                        T2, [10, 3], NCH2)

    # per-core dst-side dinv tables
    dinv_l = dinv.reshape(N_CORES, NL)
    dinv_fm = np.repeat(dinv_l[:, None, :], 128, axis=1)     # [8,128,NL]
    dinv_tiles = np.ascontiguousarray(
        dinv_l.reshape(N_CORES, T1, 128).transpose(0, 2, 1))  # [8,128,80]

    # head dst dinv: hzT[p, k, g] -> host h=2k+(p>=64), feat=p%64,
    # local node g*40+h
    dinv_hz = np.zeros((N_CORES, 128, 7, GPC), np.float32)
    for k in range(7):
        for half in range(2):
            h = 2 * k + half
            if h >= NH:
                continue
            nodes = np.arange(GPC) * GRAPH + h
            dinv_hz[:, 64 * half:64 * (half + 1), k, :] = \
                dinv_l[:, nodes][:, None, :]

    return dict(dinv=dinv, sch1=sch1, sch2=sch2, dinv_fm=dinv_fm,
                dinv_tiles=dinv_tiles, dinv_hz=dinv_hz)


# ---------------------------------------------------------------- builder

def _build(meta):
    sch1, sch2 = meta["sch1"], meta["sch2"]
    L1, L2 = sch1["L"], sch2["L"]
    nc = bacc.Bacc("TRN2", target_bir_lowering=False, debug=False,
                   num_devices=N_CORES, num_swdge_queues=4)
    d_xs = nc.dram_tensor("xs", [N, IN_DIM], BF16, kind="ExternalInput")
    d_xsl = nc.dram_tensor("xsl", [NL, IN_DIM], BF16, kind="ExternalInput")
    d_idx1 = nc.dram_tensor("idx1", [128, L1 // 16], I16, kind="ExternalInput")
    d_dstv1 = nc.dram_tensor("dstv1", [128, L1 // 128], F16,
                             kind="ExternalInput")
    d_idx2 = nc.dram_tensor("idx2", [128, L2 // 16], I16, kind="ExternalInput")
    d_dstv2 = nc.dram_tensor("dstv2", [128, L2 // 128], F16,
                             kind="ExternalInput")
    d_dinvfm = nc.dram_tensor("dinvfm", [128, NL], BF16, kind="ExternalInput")
    d_dinvt = nc.dram_tensor("dinvt", [128, T1], F32, kind="ExternalInput")
    d_dinvhz = nc.dram_tensor("dinvhz", [128, 7 * GPC], F32,
                              kind="ExternalInput")
    d_W1 = nc.dram_tensor("W1b", [IN_DIM, H1], BF16, kind="ExternalInput")
    d_b1 = nc.dram_tensor("b1p", [128, 2], F32, kind="ExternalInput")
    d_W2 = nc.dram_tensor("W2b", [128, 2 * H2], BF16, kind="ExternalInput")
    d_b2hz = nc.dram_tensor("b2hz", [128, 1], F32, kind="ExternalInput")
    d_Wout = nc.dram_tensor("Woutp", [128, 7 * ACT], F32, kind="ExternalInput")
    d_bout = nc.dram_tensor("bout", [1, ACT], F32, kind="ExternalInput")
    d_out = nc.dram_tensor("out", [GPC, ACT], F32, kind="ExternalOutput")

    qi = [0]   # global gather counter -> queue = qi % 4 (lane stays aligned)

    with tile.TileContext(nc) as tc, ExitStack() as top:
        perm = top.enter_context(tc.tile_pool(name="perm", bufs=1))
        dram = top.enter_context(tc.tile_pool(name="dram", bufs=1,
                                              space="DRAM"))

        # ---- persistent tiles (all loads via HWDGE to keep Pool clean)
        idx1t = perm.tile([128, L1 // 16], I16)
        nc.sync.dma_start(out=idx1t[:], in_=d_idx1[:])
        dstv1t = perm.tile([128, L1 // 128], F16)
        nc.sync.dma_start(out=dstv1t[:], in_=d_dstv1[:])
        idx2t = perm.tile([128, L2 // 16], I16)
        nc.sync.dma_start(out=idx2t[:], in_=d_idx2[:])
        dstv2t = perm.tile([128, L2 // 128], F16)
        nc.sync.dma_start(out=dstv2t[:], in_=d_dstv2[:])
        dinvfm = perm.tile([128, NL], BF16)
        nc.sync.dma_start(out=dinvfm[:], in_=d_dinvfm[:])
        dinvt = perm.tile([128, T1], F32)
        nc.sync.dma_start(out=dinvt[:], in_=d_dinvt[:])
        W1sb = perm.tile([128, H1], BF16)
        nc.sync.dma_start(out=W1sb[:], in_=d_W1[:])
        b1sb = perm.tile([128, 2], F32)
        nc.sync.dma_start(out=b1sb[:], in_=d_b1[:])
        W2sb = perm.tile([128, 2, H2], BF16)
        nc.sync.dma_start(out=W2sb[:].rearrange("p m f -> p (m f)"),
                          in_=d_W2[:])
        b2hz = perm.tile([128, 1], F32)
        nc.sync.dma_start(out=b2hz[:], in_=d_b2hz[:])
        WoutSB = perm.tile([128, 7, ACT], F32)
        nc.sync.dma_start(out=WoutSB[:].rearrange("p k a -> p (k a)"),
                          in_=d_Wout[:])
        boutrep = perm.tile([128, ACT], F32)
        nc.sync.dma_start(out=boutrep[:], in_=d_bout[:].to_broadcast((128, ACT)))
        dinvhz = perm.tile([128, 7, GPC], F32)
        nc.sync.dma_start(out=dinvhz[:].rearrange("p k g -> p (k g)"),
                          in_=d_dinvhz[:])

        zmm = perm.tile([128, 512], BF16)
        nc.gpsimd.memset(zmm[:], 0.0)
        ident = perm.tile([128, 128], F32)
        make_identity(nc, ident[:])
        iota_i = perm.tile([128, 256], I32)
        nc.gpsimd.iota(iota_i[:], pattern=[[1, 256]], base=0,
                       channel_multiplier=0)
        iota_bf = perm.tile([128, 256], F16)
        nc.vector.tensor_copy(out=iota_bf[:], in_=iota_i[:])
        iota_hi_i = perm.tile([128, 256], I32)
        nc.gpsimd.iota(iota_hi_i[:], pattern=[[1, 256]], base=256,
                       channel_multiplier=0)
        iota_hi = perm.tile([128, 256], F16)
        nc.vector.tensor_copy(out=iota_hi[:], in_=iota_hi_i[:])
        iota_f = perm.tile([128, 128], F32)
        nc.vector.tensor_copy(out=iota_f[:], in_=iota_i[:, 0:128])
        ioc = perm.tile([128, 1], I32)
        nc.gpsimd.iota(ioc[:], pattern=[[1, 1]], base=64, channel_multiplier=1)
        iocf = perm.tile([128, 1], F32)
        nc.vector.tensor_copy(out=iocf[:], in_=ioc[:])
        ident_hi = perm.tile([128, 128], F32)
        nc.vector.tensor_tensor(out=ident_hi[:],
                                in0=iocf[:].to_broadcast((128, 128)),
                                in1=iota_f[:], op=mybir.AluOpType.is_equal)

        m2sl = [dram.tile([AGCS[k], H2], F32, name=f"m2sl{k}")
                for k in range(NCH2)]
        m2sf = [dram.tile([AGCS[k] * N_CORES, H2], F32, addr_space="Shared",
                          name=f"m2sf{k}") for k in range(NCH2)]
        agg2h = perm.tile([64, NHOST], F32)

        # split layer-1 calls by block for AllGather interleaving
        pb1 = np.concatenate([[0], np.cumsum([x // 2 for x in BLK1S])])
        tb1 = np.concatenate([[0], np.cumsum(BLK1S)])
        calls1_by_blk = [[] for _ in range(NB1)]
        for (c, s0, n_g, evs) in sch1["calls"]:
            b = int(np.searchsorted(pb1, evs[0][1], side="right")) - 1
            calls1_by_blk[b].append((c, s0, n_g, evs))

        def gather(dst_tile, src_ap, idxt, s0, n_g, elem):
            nc.gpsimd.dma_gather(
                out_ap=dst_tile[:, 0:n_g, :],
                in_ap=src_ap,
                idxs_ap=idxt[:, s0 // 16: s0 // 16 + n_g * 8],
                num_idxs=n_g * 128, num_idxs_reg=n_g * 128,
                elem_size=elem, single_packet=False,
                queue_num=qi[0] % 4)
            qi[0] += 1

        def build_oh(wk, dstvt, s0, n_g):
            oh = wk.tile([128, CALL_G, 256], BF16, tag="oh", bufs=5)
            nc.vector.tensor_tensor(
                out=oh[:, 0:n_g, :],
                in0=dstvt[:, s0 // 128: s0 // 128 + n_g].unsqueeze(2)
                    .to_broadcast((128, n_g, 256)),
                in1=iota_bf[:].unsqueeze(1).to_broadcast((128, n_g, 256)),
                op=mybir.AluOpType.is_equal)
            return oh

        # =========================== Layer 1 ===========================
        with ExitStack() as ph1:
            wk1 = ph1.enter_context(tc.tile_pool(name="wk1", bufs=2))
            psA = ph1.enter_context(tc.tile_pool(name="psA", bufs=1,
                                                 space="PSUM"))
            psG = ph1.enter_context(tc.tile_pool(name="psG", bufs=2,
                                                 space="PSUM"))
            open_ps = {}   # super-tile (4 dst tiles per PSUM bank)

            def ps1_slice(t):
                sup = t // 4
                if sup not in open_ps:
                    ph = psA.tile([128, 512], F32, tag="agg",
                                  bufs=6, name=f"ps1s{sup}")
                    open_ps[sup] = ph
                    # start=True clears has_written for the WHOLE bank, so
                    # zero the full super once; all real events accumulate.
                    nc.tensor.matmul(out=ph[:], lhsT=zmm[:, 0:128],
                                     rhs=zmm[:], start=True, stop=False)
                return open_ps[sup][:, 128 * (t % 4):128 * (t % 4 + 1)]

            def ps1_pair(p):
                sup = p // 2
                if sup not in open_ps:
                    ps1_slice(4 * sup)      # opens + zeroes the super
                return open_ps[sup][:, 256 * (p % 2):256 * (p % 2 + 1)]

            def straddle_oh(wk, dstvt, s0, g):
                oh2 = wk.tile([128, 256], BF16, tag="oh2", bufs=3)
                col = s0 // 128 + g
                nc.vector.tensor_tensor(
                    out=oh2[:],
                    in0=dstvt[:, col:col + 1].to_broadcast((128, 256)),
                    in1=iota_hi[:], op=mybir.AluOpType.is_equal)
                return oh2[:]

            for b in range(NB1):
                for (c, s0, n_g, evs) in calls1_by_blk[b]:
                    if c < 3:
                        rows0, nrows = CH1_RANGES[c]
                        src_ap = d_xs[rows0:rows0 + nrows, :]
                    else:
                        src_ap = d_xsl[:]
                    gat = wk1.tile([128, CALL_G, IN_DIM], BF16, tag="gat",
                                   bufs=6)
                    gather(gat, src_ap, idx1t, s0, n_g, IN_DIM)
                    oh = build_oh(wk1, dstv1t, s0, n_g)
                    for (g, p, first, last, straddle) in evs:
                        rhs = (straddle_oh(wk1, dstv1t, s0, g) if straddle
                               else oh[:, g, :])
                        nc.tensor.matmul(
                            out=ps1_pair(p), lhsT=gat[:, g, :],
                            rhs=rhs, start=False, stop=last)
                # close tiles of this block: dinv scale + GEMM1 + GEMM2
                for t in range(int(tb1[b]), int(tb1[b + 1])):
                    ps = ps1_slice(t)
                    if t % 4 == 3:
                        del open_ps[t // 4]
                    aggt = wk1.tile([128, 128], BF16, tag="aggt", bufs=3)
                    nc.vector.tensor_tensor(
                        out=aggt[:], in0=ps,
                        in1=dinvfm[:, 128 * t:128 * (t + 1)],
                        op=mybir.AluOpType.mult)
                    phg = psG.tile([128, 320], F32, tag="h1g")
                    h1t = wk1.tile([128, 2, 128], BF16, tag="h1t", bufs=3)
                    for m in range(2):
                        nc.tensor.matmul(
                            out=phg[:, 128 * m:128 * (m + 1)],
                            lhsT=W1sb[:, 128 * m:128 * (m + 1)],
                            rhs=aggt[:], start=True, stop=True)
                        nc.scalar.activation(
                            out=h1t[:, m, :], in_=phg[:, 128 * m:128 * (m + 1)],
                            func=mybir.ActivationFunctionType.Relu,
                            bias=b1sb[:, m:m + 1], scale=1.0)
                    for m in range(2):
                        nc.tensor.matmul(out=phg[:, 256:320],
                                         lhsT=h1t[:, m, :],
                                         rhs=W2sb[:, m, :],
                                         start=(m == 0), stop=(m == 1))
                    tb = t - int(tb1[b])
                    if tb == 0:
                        stgb = wk1.tile([128, max(BLK1S), H2], F32,
                                        tag="stgb", bufs=2)
                    nc.scalar.activation(
                        out=stgb[:, tb, :], in_=phg[:, 256:320],
                        func=mybir.ActivationFunctionType.Copy,
                        scale=dinvt[:, t:t + 1])
                # one staging DMA per block; AllGather triggers must issue
                # from the Pool queue (walrus rejects other engines), so
                # AG_k goes after block k+1's gathers to avoid a Pool stall
                nc.sync.dma_start(
                    out=m2sl[b][:].rearrange("(t p) f -> p t f", p=128),
                    in_=stgb[:, 0:BLK1S[b], :])
                ks = [b - 1] if b >= 1 else []
                for k in ks:
                    nc.gpsimd.collective_compute(
                        "AllGather", mybir.AluOpType.bypass,
                        replica_groups=[list(range(N_CORES))],
                        ins=[m2sl[k][:].opt()],
                        outs=[m2sf[k][:].opt()])

        # =========================== Layer 2 ===========================
        with ExitStack() as ph2:
            wk2 = ph2.enter_context(tc.tile_pool(name="wk2", bufs=2))
            psA2 = ph2.enter_context(tc.tile_pool(name="psA2", bufs=1,
                                                  space="PSUM"))
            open2 = {}

            def ps2_slice(t):
                sup = t // 4
                if sup not in open2:
                    ph = psA2.tile([64, 512], F32, tag="agg2",
                                   bufs=5, name=f"ps2s{sup}")
                    open2[sup] = ph
                    nc.tensor.matmul(out=ph[:], lhsT=zmm[:, 0:64],
                                     rhs=zmm[:], start=True, stop=False)
                return open2[sup][:, 128 * (t % 4):128 * (t % 4 + 1)]

            def ps2_pair(p):
                sup = p // 2
                if sup not in open2:
                    ps2_slice(4 * sup)
                return open2[sup][:, 256 * (p % 2):256 * (p % 2 + 1)]

            NP2 = T2 // 2
            for ci2, (c, s0, n_g, evs) in enumerate(sch2["calls"]):
                if ci2 == 3:
                    nc.gpsimd.collective_compute(
                        "AllGather", mybir.AluOpType.bypass,
                        replica_groups=[list(range(N_CORES))],
                        ins=[m2sl[NB1 - 1][:].opt()],
                        outs=[m2sf[NB1 - 1][:].opt()])
                gat2 = wk2.tile([128, CALL_G, H2], F32, tag="gat2", bufs=5)
                gather(gat2, m2sf[c][:], idx2t, s0, n_g, H2)
                g2b = wk2.tile([128, CALL_G, H2], BF16, tag="g2b", bufs=5)
                nc.scalar.activation(
                    out=g2b[:, 0:n_g, :], in_=gat2[:, 0:n_g, :],
                    func=mybir.ActivationFunctionType.Copy)
                oh = build_oh(wk2, dstv2t, s0, n_g)
                for (g, p, first, last, straddle) in evs:
                    rhs = (straddle_oh(wk2, dstv2t, s0, g) if straddle
                           else oh[:, g, :])
                    nc.tensor.matmul(
                        out=ps2_pair(p), lhsT=g2b[:, g, :],
                        rhs=rhs, start=False, stop=last)
                    if last:
                        nc.scalar.activation(
                            out=agg2h[:, 256 * p:256 * (p + 1)],
                            in_=ps2_pair(p),
                            func=mybir.ActivationFunctionType.Copy)
                        if p % 2 == 1 or p == NP2 - 1:
                            del open2[p // 2]

            # ===================== actor head ==========================
            with ExitStack() as ph4:
                mid4 = ph4.enter_context(tc.tile_pool(name="mid4", bufs=1))
                wk4 = ph4.enter_context(tc.tile_pool(name="wk4", bufs=2))
                psF = ph4.enter_context(tc.tile_pool(name="psF", bufs=2,
                                                     space="PSUM"))
                hzT = mid4.tile([128, 7, GPC], F32)
                h2r = agg2h[:].rearrange("p (q g) -> p q g", q=NH)
                for k in range(7):
                    hd = psF.tile([128, 416], F32, tag="hd", name=f"hzps{k}")
                    pk = hd[:, 0:GPC]
                    nc.tensor.matmul(out=pk, lhsT=ident[0:64, :],
                                     rhs=h2r[:, 2 * k, :],
                                     start=True, stop=(k == 6))
                    if k < 6:
                        nc.tensor.matmul(out=pk, lhsT=ident_hi[0:64, :],
                                         rhs=h2r[:, 2 * k + 1, :],
                                         start=False, stop=True)
                    nc.vector.tensor_tensor(out=hzT[:, k, :], in0=pk,
                                            in1=dinvhz[:, k, :],
                                            op=mybir.AluOpType.mult)
                nc.scalar.activation(out=hzT[:].rearrange("p k g -> p (k g)"),
                                     in_=hzT[:].rearrange("p k g -> p (k g)"),
                                     func=mybir.ActivationFunctionType.Relu,
                                     bias=b2hz[:, 0:1], scale=1.0)
                for m in range(GPC // 128):
                    hdf = psF.tile([128, 416], F32, tag="hd", name=f"finps{m}")
                    pf = hdf[:, 256:256 + ACT]
                    for k in range(6):
                        nc.tensor.matmul(
                            out=pf, lhsT=hzT[:, k, 128 * m:128 * (m + 1)],
                            rhs=WoutSB[:, k, :], start=(k == 0), stop=False)
                    nc.tensor.matmul(
                        out=pf, lhsT=hzT[0:64, 6, 128 * m:128 * (m + 1)],
                        rhs=WoutSB[0:64, 6, :], start=False, stop=True)
                    nc.vector.tensor_tensor(out=pf, in0=pf,
                                            in1=boutrep[:],
                                            op=mybir.AluOpType.add)
                    mx = wk4.tile([128, 1], F32, tag="mx")
                    nc.vector.tensor_reduce(out=mx[:], in_=pf,
                                            axis=mybir.AxisListType.X,
                                            op=mybir.AluOpType.max)
                    nmx = wk4.tile([128, 1], F32, tag="nmx")
                    nc.vector.tensor_scalar_mul(nmx[:], mx[:], -1.0)
                    esb = wk4.tile([128, ACT], F32, tag="esb")
                    nc.scalar.activation(out=esb[:], in_=pf,
                                         func=mybir.ActivationFunctionType.Exp,
                                         bias=nmx[:, 0:1], scale=1.0)
                    ssum = wk4.tile([128, 1], F32, tag="ssum")
                    nc.vector.tensor_reduce(out=ssum[:], in_=esb[:],
                                            axis=mybir.AxisListType.X,
                                            op=mybir.AluOpType.add)
                    rcp = wk4.tile([128, 1], F32, tag="rcp")
                    nc.vector.reciprocal(out=rcp[:], in_=ssum[:])
                    osb = wk4.tile([128, ACT], F32, tag="osb")
                    nc.vector.tensor_scalar_mul(osb[:], esb[:], rcp[:, 0:1])
                    nc.sync.dma_start(out=d_out[128 * m:128 * (m + 1), :],
                                      in_=osb[:])

    nc.compile()
    return nc


# ---------------------------------------------------------------- entry

_CACHE = {}


def _get(ei):
    key = hashlib.sha1(ei.tobytes()).hexdigest()
    if key not in _CACHE:
        meta = _prep(ei)
        nc = _build(meta)
        _CACHE[key] = (meta, nc)
    return _CACHE[key]


def _in_maps(meta, x, W1, b1, W2, b2, Wout, bout):
    dinv = meta["dinv"]
    xs = (x.astype(np.float32) * dinv[:, None]).astype(ml_dtypes.bfloat16)
    b1p = np.ascontiguousarray(
        np.asarray(b1, np.float32).reshape(2, 128).T)            # [128,2]
    W2p = np.ascontiguousarray(
        np.asarray(W2, np.float32).reshape(2, 128, H2).transpose(1, 0, 2)
        .reshape(128, 2 * H2)).astype(ml_dtypes.bfloat16)
    Woutp = np.zeros((128, 7, ACT), np.float32)
    for k in range(6):
        Woutp[:, k, :] = Wout[128 * k:128 * (k + 1), :]
    Woutp[0:64, 6, :] = Wout[768:832, :]
    b2t = np.tile(np.asarray(b2, np.float32).reshape(H2), 2).reshape(128, 1)
    maps = []
    for r in range(N_CORES):
        maps.append({
            "xs": xs,
            "xsl": np.ascontiguousarray(xs[r * NL:(r + 1) * NL]),
            "idx1": np.ascontiguousarray(meta["sch1"]["idx_sb"][r]),
            "dstv1": np.ascontiguousarray(meta["sch1"]["dstv_sb"][r]),
            "idx2": np.ascontiguousarray(meta["sch2"]["idx_sb"][r]),
            "dstv2": np.ascontiguousarray(meta["sch2"]["dstv_sb"][r]),
            "dinvfm": np.ascontiguousarray(meta["dinv_fm"][r])
                .astype(ml_dtypes.bfloat16),
            "dinvt": np.ascontiguousarray(meta["dinv_tiles"][r]),
            "dinvhz": np.ascontiguousarray(
                meta["dinv_hz"][r].reshape(128, 7 * GPC)),
            "W1b": np.ascontiguousarray(W1).astype(ml_dtypes.bfloat16),
            "b1p": b1p,
            "W2b": W2p,
            "b2hz": b2t,
            "Woutp": np.ascontiguousarray(Woutp.reshape(128, 7 * ACT)),
            "bout": np.ascontiguousarray(bout, np.float32).reshape(1, ACT),
        })
    return maps


def kernel(x, ei, W1, b1, W2, b2, Wout, bout, _trace=False):
    x = np.ascontiguousarray(x, np.float32)
    ei = np.ascontiguousarray(ei, np.int32)
    meta, nc = _get(ei)
    maps = _in_maps(meta, x, W1, b1, W2, b2, Wout, bout)
    res = bass_utils.run_bass_kernel_spmd(
        nc, maps, core_ids=list(range(N_CORES)), trace=_trace)
    out = np.concatenate([res.results[r]["out"] for r in range(N_CORES)],
                         axis=0).astype(np.float32)
    if _trace:
        return out, res.exec_time_ns
    return out


def install_profile_hook():
    import types
    sys.path.insert(0, "/root/.axon_site")
    import trn_agent_boot.trn_boot as _tb
    import antenv
    if "antenv.axon_hooks" not in sys.modules:
        _mod = types.ModuleType("antenv.axon_hooks")
        _h = [None]
        _mod.set_axon_ntff_profile_hook = lambda h: _h.__setitem__(0, h)
        _mod.get_axon_ntff_profile_hook = lambda: _h[0]
        sys.modules["antenv.axon_hooks"] = _mod
        antenv.axon_hooks = _mod
        _mod.set_axon_ntff_profile_hook(
            _tb._ntff_profile_via_ctypes("/opt/axon/libaxon_pjrt.so"))


# revision 44
# speedup vs baseline: 1.0327x; 1.0327x over previous
"""Trainium2 Bass kernel for nn_ActorNetwork (2-layer GCN + actor head).

Self-contained: hardcodes all shapes/sharding (8 NeuronCores).

Strategy (v2):
  - Shard dst nodes (= graphs) contiguously across 8 cores (10240 nodes/core).
  - Gather sources per edge with gpsimd dma_gather, round-robined across the
    4 SWDGE queues so descriptor generation runs on all 4 Q7 core pairs.
  - Host prescales x by dinv (bf16): gathered rows feed the one-hot
    aggregation matmuls directly (no per-edge coef multiply on-chip);
    dst-side dinv applied once per 128-dst tile at PSUM close.
  - Self-loops are ordinary edges in the edge list.
  - Edges sorted by (dst-tile-block, src-chunk, dst-tile); PSUM accumulates
    across all 3 src-chunks of a 10-tile block; per-tile close fuses
    GEMM1+ReLU+GEMM2+dinv scale, streaming m2s [NL,64] f32 to DRAM.
  - AllGather of m2s split in 4 chunks (2 blocks each), overlapped with the
    remaining layer-1 edge work; layer-2 gathers chunk against the 4
    AllGather output tensors (20480 rows each, int16-indexable).
  - Layer 2 computes only host-dst rows (first 13 of each 40), compacted
    h-major: slot c = h*256 + g  (3328 rows = 26 tiles per core).
  - Head: identical to baseline modulo the h-major rearrange.
"""
import sys
import hashlib

sys.path.insert(0, "/opt/trn_rl_repo")

import numpy as np
import ml_dtypes
from contextlib import ExitStack

from concourse import bass, mybir, tile, bass_utils, bacc
from concourse.masks import make_identity

F32 = mybir.dt.float32
BF16 = mybir.dt.bfloat16
F16 = mybir.dt.float16
I16 = mybir.dt.int16
I32 = mybir.dt.int32

N_CORES = 8
N = 81920
NL = N // N_CORES          # 10240 nodes per core
IN_DIM = 128
H1 = 256
H2 = 64
GRAPH = 40
NH = 13
ACT = 145
GPC = NL // GRAPH          # 256 graphs per core
SENT = 600.0
CALL_G = 16                # groups (of 128 idxs) per dma_gather call

# layer 1 dst layout: 80 tiles of 128 local nodes, blocks of 20 tiles
# (PSUM is bank-granular: one [128, 512] f32 bank holds 4 dst tiles, so a
#  20-tile block = 5 banks of open accumulators.)
T1 = NL // 128             # 80
BLK1 = 20
NB1 = T1 // BLK1           # 4 blocks (1 AllGather chunk each)
CH1 = 32768                # src chunk rows (int16 idx) over xs [N, 128]
NCH1 = 3
CH1_RANGES = [(0, 32768), (32768, 32768), (65536, 16384)]

# layer 2 dst layout: host slots c = h*256 + g; 3328 = 26 tiles
NHOST = NH * GPC           # 3328
T2 = NHOST // 128          # 26
BLK2 = 20                  # blocks of 20,6 tiles
NB2 = (T2 + BLK2 - 1) // BLK2
# layer-2 source = 4 AllGather output tensors of AGR rows each
AGC = NL // 4              # 2560 rows contributed per core per AG chunk
AGR = AGC * N_CORES        # 20480 rows per AG tensor
NCH2 = 4


# ---------------------------------------------------------------- host prep

def _mk_schedule(core, c_of, t_of, dloc256, idxl, ntiles, blk, nch):
    """Common-max padded, block-major schedule shared by all cores.

    Segment granularity = (chunk, tile-PAIR): each 128-slot group maps to
    exactly one 256-dst window (one matmul per group, no tile spans).
    core/c_of/t_of/dloc256/idxl: per-edge arrays (dst-owning core, src chunk,
    dst tile, dst%256, chunk-local src index). blk in TILES (even).
    """
    assert ntiles % 2 == 0 and blk % 2 == 0
    npair = ntiles // 2
    blkp = blk // 2
    p_of = t_of // 2
    nblk = (npair + blkp - 1) // blkp
    counts = np.zeros((N_CORES, nch, npair), np.int64)
    np.add.at(counts, (core, c_of, p_of), 1)
    Ncm = counts.max(axis=0)                    # [nch, npair]
    assert Ncm.min() > 0, "empty (chunk, pair) segment"

    seg_off = np.zeros((nch, npair), np.int64)
    runs = []                                   # (b, c, start, ngroups)
    off = 0
    for b in range(nblk):
        plo, phi = b * blkp, min((b + 1) * blkp, npair)
        for c in range(nch):
            start = off
            for p in range(plo, phi):
                seg_off[c, p] = off
                off += int(Ncm[c, p])
            if (off - start) % 128:
                off += 128 - (off - start) % 128
            runs.append((b, c, start, (off - start) // 128))
    L = int(off)

    # groups: base pair = pair of first slot; a group whose 128 slots cross
    # the (common) segment boundary into pair+1 emits a second "straddle"
    # event (one-hot built against iota+256 at the kernel level).
    calls = []
    gpair_of_slot = np.full(L, -1, np.int64)
    for (b, c, start, ngroups) in runs:
        plo, phi = b * blkp, min((b + 1) * blkp, npair)
        segs = [(int(seg_off[c, p]), int(Ncm[c, p]), p)
                for p in range(plo, phi)]
        gev = []
        for g in range(ngroups):
            s0 = start + 128 * g
            pg = segs[-1][2]
            for (so, n, p) in segs:
                if s0 < so + n:
                    pg = p
                    break
            gpair_of_slot[s0:s0 + 128] = pg
            evs = [[g, pg, False, False, False]]
            for (so, n, p) in segs:
                if p == pg:
                    if s0 + 128 > so + n and p + 1 < phi:
                        # straddles into pair p+1
                        so2, n2, _ = segs[p + 1 - plo]
                        assert s0 + 128 <= so2 + n2, "group spans >2 pairs"
                        evs.append([g, p + 1, False, False, True])
                    break
            gev.append(evs)
        gi = 0
        while gi < ngroups:
            n = min(CALL_G, ngroups - gi)
            evs = []
            for g in range(gi, gi + n):
                for (gg, p, f, l, st) in gev[g]:
                    evs.append([gg - gi, p, f, l, st])
            calls.append([c, start + 128 * gi, n, evs])
            gi += n

    # first/last event per PAIR for psum open/close flags
    first_seen, last_seen = {}, {}
    for ci, (c, s0, n, evs) in enumerate(calls):
        for ei, ev in enumerate(evs):
            p = ev[1]
            if p not in first_seen:
                first_seen[p] = (ci, ei)
            last_seen[p] = (ci, ei)
    for p, (ci, ei) in first_seen.items():
        calls[ci][3][ei][2] = True
    for p, (ci, ei) in last_seen.items():
        calls[ci][3][ei][3] = True
    assert len(first_seen) == npair

    idx_all = np.zeros((N_CORES, L), np.int16)
    dstv_all = np.full((N_CORES, L), SENT, np.float32)
    for r in range(N_CORES):
        m = core == r
        sc, sp = c_of[m], p_of[m]
        sd, si = dloc256[m], idxl[m]
        key = sc * npair + sp
        order = np.lexsort((key,))
        sc, sp, sd, si = sc[order], sp[order], sd[order], si[order]
        key = key[order]
        change = np.r_[True, key[1:] != key[:-1]]
        starts = np.flatnonzero(change)
        runid = np.cumsum(change) - 1
        within = np.arange(len(key)) - starts[runid]
        pos = seg_off[sc, sp] + within
        idx_all[r, pos] = si.astype(np.int16)
        # dstv relative to the slot's GROUP base pair: [0,256) for the base
        # pair, [256,512) for the next pair (straddle window)
        rel = sd + 256.0 * (sp - gpair_of_slot[pos])
        assert rel.min() >= 0 and rel.max() < 512
        dstv_all[r, pos] = rel

    idx_sb = np.stack([
        np.tile(idx_all[r].reshape(-1, 16).T, (8, 1)) for r in range(N_CORES)
    ])                                          # [8, 128, L/16]
    dstv_sb = np.stack([
        dstv_all[r].reshape(-1, 128).T for r in range(N_CORES)
    ]).astype(np.float16)                       # [8, 128, L/128] (ints exact)
    return dict(L=L, calls=calls, idx_sb=idx_sb, dstv_sb=dstv_sb)


def _prep(ei):
    src = ei[0].astype(np.int64)
    dst = ei[1].astype(np.int64)
    deg = np.bincount(dst, minlength=N).astype(np.float64) + 1.0
    dinv = (1.0 / np.sqrt(deg)).astype(np.float32)

    # ---------------- layer 1: random edges only (self loops are applied
    # as sequential HWDGE loads + transpose-matmuls in the kernel)
    all_n = np.arange(N, dtype=np.int64)
    core1 = dst // NL
    du1 = dst % NL
    sch1 = _mk_schedule(core1, src // CH1, du1 // 128,
                        (du1 % 256).astype(np.float32), src % CH1,
                        T1, BLK1, NCH1)

    # ---------------- layer 2: host-dst edges + host self loops
    hm = (dst % GRAPH) < NH
    s2 = src[hm]
    d2 = dst[hm]
    core2 = d2 // NL
    nloc = d2 % NL
    g2 = nloc // GRAPH
    h2 = nloc % GRAPH
    c2 = h2 * GPC + g2                          # compacted host slot
    # m2sf position: src s = r*NL + n -> AG tensor a = n//AGC,
    # row = (s//NL)*AGC + n%AGC
    sn = s2 % NL
    a2 = sn // AGC
    pos2 = (s2 // NL) * AGC + (sn % AGC)
    sch2 = _mk_schedule(core2, a2, c2 // 128,
                        (c2 % 256).astype(np.float32), pos2,
                        T2, BLK2, NCH2)

    # per-core dst-side dinv tables
    dinv_l = dinv.reshape(N_CORES, NL)
    dinv_fm = np.repeat(dinv_l[:, None, :], 128, axis=1)     # [8,128,NL]
    dinv_tiles = np.ascontiguousarray(
        dinv_l.reshape(N_CORES, T1, 128).transpose(0, 2, 1))  # [8,128,80]

    # head dst dinv: hzT[p, k, g] -> host h=2k+(p>=64), feat=p%64,
    # local node g*40+h
    dinv_hz = np.zeros((N_CORES, 128, 7, GPC), np.float32)
    for k in range(7):
        for half in range(2):
            h = 2 * k + half
            if h >= NH:
                continue
            nodes = np.arange(GPC) * GRAPH + h
            dinv_hz[:, 64 * half:64 * (half + 1), k, :] = \
                dinv_l[:, nodes][:, None, :]

    return dict(dinv=dinv, sch1=sch1, sch2=sch2, dinv_fm=dinv_fm,
                dinv_tiles=dinv_tiles, dinv_hz=dinv_hz)


# ---------------------------------------------------------------- builder

def _build(meta):
    sch1, sch2 = meta["sch1"], meta["sch2"]
    L1, L2 = sch1["L"], sch2["L"]
    nc = bacc.Bacc("TRN2", target_bir_lowering=False, debug=False,
                   num_devices=N_CORES, num_swdge_queues=4)
    d_xs = nc.dram_tensor("xs", [N, IN_DIM], BF16, kind="ExternalInput")
    d_xsl = nc.dram_tensor("xsl", [NL, IN_DIM], BF16, kind="ExternalInput")
    d_idx1 = nc.dram_tensor("idx1", [128, L1 // 16], I16, kind="ExternalInput")
    d_dstv1 = nc.dram_tensor("dstv1", [128, L1 // 128], F16,
                             kind="ExternalInput")
    d_idx2 = nc.dram_tensor("idx2", [128, L2 // 16], I16, kind="ExternalInput")
    d_dstv2 = nc.dram_tensor("dstv2", [128, L2 // 128], F16,
                             kind="ExternalInput")
    d_dinvfm = nc.dram_tensor("dinvfm", [128, NL], BF16, kind="ExternalInput")
    d_dinvt = nc.dram_tensor("dinvt", [128, T1], F32, kind="ExternalInput")
    d_dinvhz = nc.dram_tensor("dinvhz", [128, 7 * GPC], F32,
                              kind="ExternalInput")
    d_W1 = nc.dram_tensor("W1b", [IN_DIM, H1], BF16, kind="ExternalInput")
    d_b1 = nc.dram_tensor("b1p", [128, 2], F32, kind="ExternalInput")
    d_W2 = nc.dram_tensor("W2b", [128, 2 * H2], BF16, kind="ExternalInput")
    d_b2hz = nc.dram_tensor("b2hz", [128, 1], F32, kind="ExternalInput")
    d_Wout = nc.dram_tensor("Woutp", [128, 7 * ACT], F32, kind="ExternalInput")
    d_bout = nc.dram_tensor("bout", [1, ACT], F32, kind="ExternalInput")
    d_out = nc.dram_tensor("out", [GPC, ACT], F32, kind="ExternalOutput")

    qi = [0]   # global gather counter -> queue = qi % 4 (lane stays aligned)

    with tile.TileContext(nc) as tc, ExitStack() as top:
        perm = top.enter_context(tc.tile_pool(name="perm", bufs=1))
        dram = top.enter_context(tc.tile_pool(name="dram", bufs=1,
                                              space="DRAM"))

        # ---- persistent tiles (all loads via HWDGE to keep Pool clean)
        idx1t = perm.tile([128, L1 // 16], I16)
        nc.sync.dma_start(out=idx1t[:], in_=d_idx1[:])
        dstv1t = perm.tile([128, L1 // 128], F16)
        nc.sync.dma_start(out=dstv1t[:], in_=d_dstv1[:])
        idx2t = perm.tile([128, L2 // 16], I16)
        nc.sync.dma_start(out=idx2t[:], in_=d_idx2[:])
        dstv2t = perm.tile([128, L2 // 128], F16)
        nc.sync.dma_start(out=dstv2t[:], in_=d_dstv2[:])
        dinvfm = perm.tile([128, NL], BF16)
        nc.sync.dma_start(out=dinvfm[:], in_=d_dinvfm[:])
        dinvt = perm.tile([128, T1], F32)
        nc.sync.dma_start(out=dinvt[:], in_=d_dinvt[:])
        W1sb = perm.tile([128, H1], BF16)
        nc.sync.dma_start(out=W1sb[:], in_=d_W1[:])
        b1sb = perm.tile([128, 2], F32)
        nc.sync.dma_start(out=b1sb[:], in_=d_b1[:])
        W2sb = perm.tile([128, 2, H2], BF16)
        nc.sync.dma_start(out=W2sb[:].rearrange("p m f -> p (m f)"),
                          in_=d_W2[:])
        b2hz = perm.tile([128, 1], F32)
        nc.sync.dma_start(out=b2hz[:], in_=d_b2hz[:])
        WoutSB = perm.tile([128, 7, ACT], F32)
        nc.sync.dma_start(out=WoutSB[:].rearrange("p k a -> p (k a)"),
                          in_=d_Wout[:])
        boutrep = perm.tile([128, ACT], F32)
        nc.sync.dma_start(out=boutrep[:], in_=d_bout[:].to_broadcast((128, ACT)))
        dinvhz = perm.tile([128, 7, GPC], F32)
        nc.sync.dma_start(out=dinvhz[:].rearrange("p k g -> p (k g)"),
                          in_=d_dinvhz[:])

        zmm = perm.tile([128, 512], BF16)
        nc.gpsimd.memset(zmm[:], 0.0)
        ident = perm.tile([128, 128], F32)
        make_identity(nc, ident[:])
        identb = perm.tile([128, 128], BF16)
        nc.vector.tensor_copy(out=identb[:], in_=ident[:])
        iota_i = perm.tile([128, 256], I32)
        nc.gpsimd.iota(iota_i[:], pattern=[[1, 256]], base=0,
                       channel_multiplier=0)
        iota_bf = perm.tile([128, 256], F16)
        nc.vector.tensor_copy(out=iota_bf[:], in_=iota_i[:])
        iota_hi_i = perm.tile([128, 256], I32)
        nc.gpsimd.iota(iota_hi_i[:], pattern=[[1, 256]], base=256,
                       channel_multiplier=0)
        iota_hi = perm.tile([128, 256], F16)
        nc.vector.tensor_copy(out=iota_hi[:], in_=iota_hi_i[:])
        iota_f = perm.tile([128, 128], F32)
        nc.vector.tensor_copy(out=iota_f[:], in_=iota_i[:, 0:128])
        ioc = perm.tile([128, 1], I32)
        nc.gpsimd.iota(ioc[:], pattern=[[1, 1]], base=64, channel_multiplier=1)
        iocf = perm.tile([128, 1], F32)
        nc.vector.tensor_copy(out=iocf[:], in_=ioc[:])
        ident_hi = perm.tile([128, 128], F32)
        nc.vector.tensor_tensor(out=ident_hi[:],
                                in0=iocf[:].to_broadcast((128, 128)),
                                in1=iota_f[:], op=mybir.AluOpType.is_equal)

        m2sl = [dram.tile([AGC, H2], F32, name=f"m2sl{k}")
                for k in range(NCH2)]
        m2sf = [dram.tile([AGR, H2], F32, addr_space="Shared",
                          name=f"m2sf{k}") for k in range(NCH2)]
        agg2h = perm.tile([64, NHOST], F32)

        # split layer-1 calls by block for AllGather interleaving
        calls1_by_blk = [[] for _ in range(NB1)]
        for (c, s0, n_g, evs) in sch1["calls"]:
            b = evs[0][1] // (BLK1 // 2)
            calls1_by_blk[b].append((c, s0, n_g, evs))

        def gather(dst_tile, src_ap, idxt, s0, n_g, elem):
            nc.gpsimd.dma_gather(
                out_ap=dst_tile[:, 0:n_g, :],
                in_ap=src_ap,
                idxs_ap=idxt[:, s0 // 16: s0 // 16 + n_g * 8],
                num_idxs=n_g * 128, num_idxs_reg=n_g * 128,
                elem_size=elem, single_packet=False,
                queue_num=qi[0] % 4)
            qi[0] += 1

        def build_oh(wk, dstvt, s0, n_g):
            oh = wk.tile([128, CALL_G, 256], BF16, tag="oh", bufs=5)
            nc.vector.tensor_tensor(
                out=oh[:, 0:n_g, :],
                in0=dstvt[:, s0 // 128: s0 // 128 + n_g].unsqueeze(2)
                    .to_broadcast((128, n_g, 256)),
                in1=iota_bf[:].unsqueeze(1).to_broadcast((128, n_g, 256)),
                op=mybir.AluOpType.is_equal)
            return oh

        # =========================== Layer 1 ===========================
        with ExitStack() as ph1:
            wk1 = ph1.enter_context(tc.tile_pool(name="wk1", bufs=2))
            psA = ph1.enter_context(tc.tile_pool(name="psA", bufs=1,
                                                 space="PSUM"))
            psG = ph1.enter_context(tc.tile_pool(name="psG", bufs=2,
                                                 space="PSUM"))
            open_ps = {}   # super-tile (4 dst tiles per PSUM bank)

            def ps1_slice(t):
                sup = t // 4
                if sup not in open_ps:
                    ph = psA.tile([128, 512], F32, tag="agg",
                                  bufs=5, name=f"ps1s{sup}")
                    open_ps[sup] = ph
                    # start=True clears has_written for the WHOLE bank, so
                    # zero the full super once; all real events accumulate.
                    nc.tensor.matmul(out=ph[:], lhsT=zmm[:, 0:128],
                                     rhs=zmm[:], start=True, stop=False)
                return open_ps[sup][:, 128 * (t % 4):128 * (t % 4 + 1)]

            def ps1_pair(p):
                sup = p // 2
                if sup not in open_ps:
                    ps1_slice(4 * sup)      # opens + zeroes the super
                return open_ps[sup][:, 256 * (p % 2):256 * (p % 2 + 1)]

            def straddle_oh(wk, dstvt, s0, g):
                oh2 = wk.tile([128, 256], BF16, tag="oh2", bufs=3)
                col = s0 // 128 + g
                nc.vector.tensor_tensor(
                    out=oh2[:],
                    in0=dstvt[:, col:col + 1].to_broadcast((128, 256)),
                    in1=iota_hi[:], op=mybir.AluOpType.is_equal)
                return oh2[:]

            for b in range(NB1):
                for (c, s0, n_g, evs) in calls1_by_blk[b]:
                    rows0, nrows = CH1_RANGES[c]
                    src_ap = d_xs[rows0:rows0 + nrows, :]
                    gat = wk1.tile([128, CALL_G, IN_DIM], BF16, tag="gat",
                                   bufs=6)
                    gather(gat, src_ap, idx1t, s0, n_g, IN_DIM)
                    oh = build_oh(wk1, dstv1t, s0, n_g)
                    for (g, p, first, last, straddle) in evs:
                        rhs = (straddle_oh(wk1, dstv1t, s0, g) if straddle
                               else oh[:, g, :])
                        nc.tensor.matmul(
                            out=ps1_pair(p), lhsT=gat[:, g, :],
                            rhs=rhs, start=False, stop=False)
                # self loops: sequential xsl tiles, transposed into psum
                for t in range(b * BLK1, (b + 1) * BLK1):
                    xslt = wk1.tile([128, 128], BF16, tag="xslt", bufs=3)
                    nc.sync.dma_start(out=xslt[:],
                                      in_=d_xsl[128 * t:128 * (t + 1), :])
                    nc.tensor.matmul(out=ps1_slice(t), lhsT=xslt[:],
                                     rhs=identb[:], start=False, stop=True)
                # close tiles of this block: dinv scale + GEMM1 + GEMM2
                for t in range(b * BLK1, (b + 1) * BLK1):
                    ps = ps1_slice(t)
                    if t % 4 == 3:
                        del open_ps[t // 4]
                    aggt = wk1.tile([128, 128], BF16, tag="aggt", bufs=3)
                    nc.vector.tensor_tensor(
                        out=aggt[:], in0=ps,
                        in1=dinvfm[:, 128 * t:128 * (t + 1)],
                        op=mybir.AluOpType.mult)
                    phg = psG.tile([128, 320], F32, tag="h1g")
                    h1t = wk1.tile([128, 2, 128], BF16, tag="h1t", bufs=3)
                    for m in range(2):
                        nc.tensor.matmul(
                            out=phg[:, 128 * m:128 * (m + 1)],
                            lhsT=W1sb[:, 128 * m:128 * (m + 1)],
                            rhs=aggt[:], start=True, stop=True)
                        nc.scalar.activation(
                            out=h1t[:, m, :], in_=phg[:, 128 * m:128 * (m + 1)],
                            func=mybir.ActivationFunctionType.Relu,
                            bias=b1sb[:, m:m + 1], scale=1.0)
                    for m in range(2):
                        nc.tensor.matmul(out=phg[:, 256:320],
                                         lhsT=h1t[:, m, :],
                                         rhs=W2sb[:, m, :],
                                         start=(m == 0), stop=(m == 1))
                    if t % BLK1 == 0:
                        stgb = wk1.tile([128, BLK1, H2], F32, tag="stgb",
                                        bufs=2)
                    nc.scalar.activation(
                        out=stgb[:, t % BLK1, :], in_=phg[:, 256:320],
                        func=mybir.ActivationFunctionType.Copy,
                        scale=dinvt[:, t:t + 1])
                # one staging DMA per block; AllGather triggers must issue
                # from the Pool queue (walrus rejects other engines), so
                # AG_k goes after block k+1's gathers to avoid a Pool stall
                nc.sync.dma_start(
                    out=m2sl[b][:].rearrange("(t p) f -> p t f", p=128),
                    in_=stgb[:])
                ks = [b - 1] if b >= 1 else []
                for k in ks:
                    nc.gpsimd.collective_compute(
                        "AllGather", mybir.AluOpType.bypass,
                        replica_groups=[list(range(N_CORES))],
                        ins=[m2sl[k][:].opt()],
                        outs=[m2sf[k][:].opt()])

        # =========================== Layer 2 ===========================
        with ExitStack() as ph2:
            wk2 = ph2.enter_context(tc.tile_pool(name="wk2", bufs=2))
            psA2 = ph2.enter_context(tc.tile_pool(name="psA2", bufs=1,
                                                  space="PSUM"))
            open2 = {}

            def ps2_slice(t):
                sup = t // 4
                if sup not in open2:
                    ph = psA2.tile([64, 512], F32, tag="agg2",
                                   bufs=5, name=f"ps2s{sup}")
                    open2[sup] = ph
                    nc.tensor.matmul(out=ph[:], lhsT=zmm[:, 0:64],
                                     rhs=zmm[:], start=True, stop=False)
                return open2[sup][:, 128 * (t % 4):128 * (t % 4 + 1)]

            def ps2_pair(p):
                sup = p // 2
                if sup not in open2:
                    ps2_slice(4 * sup)
                return open2[sup][:, 256 * (p % 2):256 * (p % 2 + 1)]

            # self rows: m2s of own host nodes, loaded from m2sl with
            # 52 small patterned HWDGE DMAs into selfh [128, T2, 64]
            selfh = perm.tile([128, T2, H2], F32)
            for a in range(4):
                rows = m2sl[a][:].rearrange("(g q) f -> g q f", q=GRAPH)
                for h in range(NH):
                    nc.sync.dma_start(
                        out=selfh[64 * (a % 2):64 * (a % 2) + 64,
                                  2 * h + a // 2, :],
                        in_=rows[:, h, :])
            NP2 = T2 // 2
            pb2 = [0, 10, 13]
            calls2_by_blk = [[], []]
            for (c, s0, n_g, evs) in sch2["calls"]:
                b2 = 0 if evs[0][1] < 10 else 1
                calls2_by_blk[b2].append((c, s0, n_g, evs))
            ci2 = 0
            for b2 in range(2):
                for (c, s0, n_g, evs) in calls2_by_blk[b2]:
                    if ci2 == 3:
                        nc.gpsimd.collective_compute(
                            "AllGather", mybir.AluOpType.bypass,
                            replica_groups=[list(range(N_CORES))],
                            ins=[m2sl[NB1 - 1][:].opt()],
                            outs=[m2sf[NB1 - 1][:].opt()])
                    ci2 += 1
                    gat2 = wk2.tile([128, CALL_G, H2], F32, tag="gat2",
                                    bufs=5)
                    gather(gat2, m2sf[c][:], idx2t, s0, n_g, H2)
                    g2b = wk2.tile([128, CALL_G, H2], BF16, tag="g2b", bufs=5)
                    nc.scalar.activation(
                        out=g2b[:, 0:n_g, :], in_=gat2[:, 0:n_g, :],
                        func=mybir.ActivationFunctionType.Copy)
                    oh = build_oh(wk2, dstv2t, s0, n_g)
                    for (g, p, first, last, straddle) in evs:
                        rhs = (straddle_oh(wk2, dstv2t, s0, g) if straddle
                               else oh[:, g, :])
                        nc.tensor.matmul(
                            out=ps2_pair(p), lhsT=g2b[:, g, :],
                            rhs=rhs, start=False, stop=False)
                # block's pairs: self matmuls then closes
                for p in range(pb2[b2], pb2[b2 + 1]):
                    for t in (2 * p, 2 * p + 1):
                        nc.tensor.matmul(out=ps2_slice(t),
                                         lhsT=selfh[:, t, :],
                                         rhs=ident[:], start=False,
                                         stop=(t % 2 == 1))
                    nc.scalar.activation(
                        out=agg2h[:, 256 * p:256 * (p + 1)],
                        in_=ps2_pair(p),
                        func=mybir.ActivationFunctionType.Copy)
                    if p % 2 == 1 or p == NP2 - 1:
                        del open2[p // 2]

            # ===================== actor head ==========================
            with ExitStack() as ph4:
                mid4 = ph4.enter_context(tc.tile_pool(name="mid4", bufs=1))
                wk4 = ph4.enter_context(tc.tile_pool(name="wk4", bufs=2))
                psF = ph4.enter_context(tc.tile_pool(name="psF", bufs=2,
                                                     space="PSUM"))
                hzT = mid4.tile([128, 7, GPC], F32)
                h2r = agg2h[:].rearrange("p (q g) -> p q g", q=NH)
                for k in range(7):
                    hd = psF.tile([128, 416], F32, tag="hd", name=f"hzps{k}")
                    pk = hd[:, 0:GPC]
                    nc.tensor.matmul(out=pk, lhsT=ident[0:64, :],
                                     rhs=h2r[:, 2 * k, :],
                                     start=True, stop=(k == 6))
                    if k < 6:
                        nc.tensor.matmul(out=pk, lhsT=ident_hi[0:64, :],
                                         rhs=h2r[:, 2 * k + 1, :],
                                         start=False, stop=True)
                    nc.vector.tensor_tensor(out=hzT[:, k, :], in0=pk,
                                            in1=dinvhz[:, k, :],
                                            op=mybir.AluOpType.mult)
                nc.scalar.activation(out=hzT[:].rearrange("p k g -> p (k g)"),
                                     in_=hzT[:].rearrange("p k g -> p (k g)"),
                                     func=mybir.ActivationFunctionType.Relu,
                                     bias=b2hz[:, 0:1], scale=1.0)
                for m in range(GPC // 128):
                    hdf = psF.tile([128, 416], F32, tag="hd", name=f"finps{m}")
                    pf = hdf[:, 256:256 + ACT]
                    for k in range(6):
                        nc.tensor.matmul(
                            out=pf, lhsT=hzT[:, k, 128 * m:128 * (m + 1)],
                            rhs=WoutSB[:, k, :], start=(k == 0), stop=False)
                    nc.tensor.matmul(
                        out=pf, lhsT=hzT[0:64, 6, 128 * m:128 * (m + 1)],
                        rhs=WoutSB[0:64, 6, :], start=False, stop=True)
                    nc.vector.tensor_tensor(out=pf, in0=pf,
                                            in1=boutrep[:],
                                            op=mybir.AluOpType.add)
                    mx = wk4.tile([128, 1], F32, tag="mx")
                    nc.vector.tensor_reduce(out=mx[:], in_=pf,
                                            axis=mybir.AxisListType.X,
                                            op=mybir.AluOpType.max)
                    nmx = wk4.tile([128, 1], F32, tag="nmx")
                    nc.vector.tensor_scalar_mul(nmx[:], mx[:], -1.0)
                    esb = wk4.tile([128, ACT], F32, tag="esb")
                    nc.scalar.activation(out=esb[:], in_=pf,
                                         func=mybir.ActivationFunctionType.Exp,
                                         bias=nmx[:, 0:1], scale=1.0)
                    ssum = wk4.tile([128, 1], F32, tag="ssum")
                    nc.vector.tensor_reduce(out=ssum[:], in_=esb[:],
                                            axis=mybir.AxisListType.X,
                                            op=mybir.AluOpType.add)
                    rcp = wk4.tile([128, 1], F32, tag="rcp")
                    nc.vector.reciprocal(out=rcp[:], in_=ssum[:])
                    osb = wk4.tile([128, ACT], F32, tag="osb")
                    nc.vector.tensor_scalar_mul(osb[:], esb[:], rcp[:, 0:1])
                    nc.sync.dma_start(out=d_out[128 * m:128 * (m + 1), :],
                                      in_=osb[:])

    nc.compile()
    return nc


# ---------------------------------------------------------------- entry

_CACHE = {}


def _get(ei):
    key = hashlib.sha1(ei.tobytes()).hexdigest()
    if key not in _CACHE:
        meta = _prep(ei)
        nc = _build(meta)
        _CACHE[key] = (meta, nc)
    return _CACHE[key]


def _in_maps(meta, x, W1, b1, W2, b2, Wout, bout):
    dinv = meta["dinv"]
    xs = (x.astype(np.float32) * dinv[:, None]).astype(ml_dtypes.bfloat16)
    b1p = np.ascontiguousarray(
        np.asarray(b1, np.float32).reshape(2, 128).T)            # [128,2]
    W2p = np.ascontiguousarray(
        np.asarray(W2, np.float32).reshape(2, 128, H2).transpose(1, 0, 2)
        .reshape(128, 2 * H2)).astype(ml_dtypes.bfloat16)
    Woutp = np.zeros((128, 7, ACT), np.float32)
    for k in range(6):
        Woutp[:, k, :] = Wout[128 * k:128 * (k + 1), :]
    Woutp[0:64, 6, :] = Wout[768:832, :]
    b2t = np.tile(np.asarray(b2, np.float32).reshape(H2), 2).reshape(128, 1)
    maps = []
    for r in range(N_CORES):
        maps.append({
            "xs": xs,
            "xsl": np.ascontiguousarray(xs[r * NL:(r + 1) * NL]),
            "idx1": np.ascontiguousarray(meta["sch1"]["idx_sb"][r]),
            "dstv1": np.ascontiguousarray(meta["sch1"]["dstv_sb"][r]),
            "idx2": np.ascontiguousarray(meta["sch2"]["idx_sb"][r]),
            "dstv2": np.ascontiguousarray(meta["sch2"]["dstv_sb"][r]),
            "dinvfm": np.ascontiguousarray(meta["dinv_fm"][r])
                .astype(ml_dtypes.bfloat16),
            "dinvt": np.ascontiguousarray(meta["dinv_tiles"][r]),
            "dinvhz": np.ascontiguousarray(
                meta["dinv_hz"][r].reshape(128, 7 * GPC)),
            "W1b": np.ascontiguousarray(W1).astype(ml_dtypes.bfloat16),
            "b1p": b1p,
            "W2b": W2p,
            "b2hz": b2t,
            "Woutp": np.ascontiguousarray(Woutp.reshape(128, 7 * ACT)),
            "bout": np.ascontiguousarray(bout, np.float32).reshape(1, ACT),
        })
    return maps


def kernel(x, ei, W1, b1, W2, b2, Wout, bout, _trace=False):
    x = np.ascontiguousarray(x, np.float32)
    ei = np.ascontiguousarray(ei, np.int32)
    meta, nc = _get(ei)
    maps = _in_maps(meta, x, W1, b1, W2, b2, Wout, bout)
    res = bass_utils.run_bass_kernel_spmd(
        nc, maps, core_ids=list(range(N_CORES)), trace=_trace)
    out = np.concatenate([res.results[r]["out"] for r in range(N_CORES)],
                         axis=0).astype(np.float32)
    if _trace:
        return out, res.exec_time_ns
    return out


def install_profile_hook():
    import types
    sys.path.insert(0, "/root/.axon_site")
    import trn_agent_boot.trn_boot as _tb
    import antenv
    if "antenv.axon_hooks" not in sys.modules:
        _mod = types.ModuleType("antenv.axon_hooks")
        _h = [None]
        _mod.set_axon_ntff_profile_hook = lambda h: _h.__setitem__(0, h)
        _mod.get_axon_ntff_profile_hook = lambda: _h[0]
        sys.modules["antenv.axon_hooks"] = _mod
        antenv.axon_hooks = _mod
        _mod.set_axon_ntff_profile_hook(
            _tb._ntff_profile_via_ctypes("/opt/axon/libaxon_pjrt.so"))


# revision 45
# speedup vs baseline: 1.0404x; 1.0074x over previous
"""Trainium2 Bass kernel for nn_ActorNetwork (2-layer GCN + actor head).

Self-contained: hardcodes all shapes/sharding (8 NeuronCores).

Strategy (v2):
  - Shard dst nodes (= graphs) contiguously across 8 cores (10240 nodes/core).
  - Gather sources per edge with gpsimd dma_gather, round-robined across the
    4 SWDGE queues so descriptor generation runs on all 4 Q7 core pairs.
  - Host prescales x by dinv (bf16): gathered rows feed the one-hot
    aggregation matmuls directly (no per-edge coef multiply on-chip);
    dst-side dinv applied once per 128-dst tile at PSUM close.
  - Self-loops are ordinary edges in the edge list.
  - Edges sorted by (dst-tile-block, src-chunk, dst-tile); PSUM accumulates
    across all 3 src-chunks of a 10-tile block; per-tile close fuses
    GEMM1+ReLU+GEMM2+dinv scale, streaming m2s [NL,64] f32 to DRAM.
  - AllGather of m2s split in 4 chunks (2 blocks each), overlapped with the
    remaining layer-1 edge work; layer-2 gathers chunk against the 4
    AllGather output tensors (20480 rows each, int16-indexable).
  - Layer 2 computes only host-dst rows (first 13 of each 40), compacted
    h-major: slot c = h*256 + g  (3328 rows = 26 tiles per core).
  - Head: identical to baseline modulo the h-major rearrange.
"""
import sys
import hashlib

sys.path.insert(0, "/opt/trn_rl_repo")

import numpy as np
import ml_dtypes
from contextlib import ExitStack

from concourse import bass, mybir, tile, bass_utils, bacc
from concourse.masks import make_identity

F32 = mybir.dt.float32
BF16 = mybir.dt.bfloat16
F16 = mybir.dt.float16
I16 = mybir.dt.int16
I32 = mybir.dt.int32

N_CORES = 8
N = 81920
NL = N // N_CORES          # 10240 nodes per core
IN_DIM = 128
H1 = 256
H2 = 64
GRAPH = 40
NH = 13
ACT = 145
GPC = NL // GRAPH          # 256 graphs per core
SENT = 600.0
CALL_G = 16                # groups (of 128 idxs) per dma_gather call

# layer 1 dst layout: 80 tiles of 128 local nodes, blocks of 20 tiles
# (PSUM is bank-granular: one [128, 512] f32 bank holds 4 dst tiles, so a
#  20-tile block = 5 banks of open accumulators.)
T1 = NL // 128             # 80
BLK1 = 20
NB1 = T1 // BLK1           # 4 blocks (1 AllGather chunk each)
CH1 = 32768                # src chunk rows (int16 idx) over xs [N, 128]
NCH1 = 4                   # 3 chunks of xs + 1 per-core self-loop chunk (xsl)
CH1_RANGES = [(0, 32768), (32768, 32768), (65536, 16384)]

# layer 2 dst layout: host slots c = h*256 + g; 3328 = 26 tiles
NHOST = NH * GPC           # 3328
T2 = NHOST // 128          # 26
BLK2 = 20                  # blocks of 20,6 tiles
NB2 = (T2 + BLK2 - 1) // BLK2
# layer-2 source = 4 AllGather output tensors of AGR rows each
AGC = NL // 4              # 2560 rows contributed per core per AG chunk
AGR = AGC * N_CORES        # 20480 rows per AG tensor
NCH2 = 4


# ---------------------------------------------------------------- host prep

def _mk_schedule(core, c_of, t_of, dloc256, idxl, ntiles, blk, nch):
    """Common-max padded, block-major schedule shared by all cores.

    Segment granularity = (chunk, tile-PAIR): each 128-slot group maps to
    exactly one 256-dst window (one matmul per group, no tile spans).
    core/c_of/t_of/dloc256/idxl: per-edge arrays (dst-owning core, src chunk,
    dst tile, dst%256, chunk-local src index). blk in TILES (even).
    """
    assert ntiles % 2 == 0 and blk % 2 == 0
    npair = ntiles // 2
    blkp = blk // 2
    p_of = t_of // 2
    nblk = (npair + blkp - 1) // blkp
    counts = np.zeros((N_CORES, nch, npair), np.int64)
    np.add.at(counts, (core, c_of, p_of), 1)
    Ncm = counts.max(axis=0)                    # [nch, npair]
    assert Ncm.min() > 0, "empty (chunk, pair) segment"

    seg_off = np.zeros((nch, npair), np.int64)
    runs = []                                   # (b, c, start, ngroups)
    off = 0
    for b in range(nblk):
        plo, phi = b * blkp, min((b + 1) * blkp, npair)
        for c in range(nch):
            start = off
            for p in range(plo, phi):
                seg_off[c, p] = off
                off += int(Ncm[c, p])
            if (off - start) % 128:
                off += 128 - (off - start) % 128
            runs.append((b, c, start, (off - start) // 128))
    L = int(off)

    # groups: base pair = pair of first slot; a group whose 128 slots cross
    # the (common) segment boundary into pair+1 emits a second "straddle"
    # event (one-hot built against iota+256 at the kernel level).
    calls = []
    gpair_of_slot = np.full(L, -1, np.int64)
    for (b, c, start, ngroups) in runs:
        plo, phi = b * blkp, min((b + 1) * blkp, npair)
        segs = [(int(seg_off[c, p]), int(Ncm[c, p]), p)
                for p in range(plo, phi)]
        gev = []
        for g in range(ngroups):
            s0 = start + 128 * g
            pg = segs[-1][2]
            for (so, n, p) in segs:
                if s0 < so + n:
                    pg = p
                    break
            gpair_of_slot[s0:s0 + 128] = pg
            evs = [[g, pg, False, False, False]]
            for (so, n, p) in segs:
                if p == pg:
                    if s0 + 128 > so + n and p + 1 < phi:
                        # straddles into pair p+1
                        so2, n2, _ = segs[p + 1 - plo]
                        assert s0 + 128 <= so2 + n2, "group spans >2 pairs"
                        evs.append([g, p + 1, False, False, True])
                    break
            gev.append(evs)
        gi = 0
        while gi < ngroups:
            n = min(CALL_G, ngroups - gi)
            evs = []
            for g in range(gi, gi + n):
                for (gg, p, f, l, st) in gev[g]:
                    evs.append([gg - gi, p, f, l, st])
            calls.append([c, start + 128 * gi, n, evs])
            gi += n

    # first/last event per PAIR for psum open/close flags
    first_seen, last_seen = {}, {}
    for ci, (c, s0, n, evs) in enumerate(calls):
        for ei, ev in enumerate(evs):
            p = ev[1]
            if p not in first_seen:
                first_seen[p] = (ci, ei)
            last_seen[p] = (ci, ei)
    for p, (ci, ei) in first_seen.items():
        calls[ci][3][ei][2] = True
    for p, (ci, ei) in last_seen.items():
        calls[ci][3][ei][3] = True
    assert len(first_seen) == npair

    idx_all = np.zeros((N_CORES, L), np.int16)
    dstv_all = np.full((N_CORES, L), SENT, np.float32)
    for r in range(N_CORES):
        m = core == r
        sc, sp = c_of[m], p_of[m]
        sd, si = dloc256[m], idxl[m]
        key = sc * npair + sp
        order = np.lexsort((key,))
        sc, sp, sd, si = sc[order], sp[order], sd[order], si[order]
        key = key[order]
        change = np.r_[True, key[1:] != key[:-1]]
        starts = np.flatnonzero(change)
        runid = np.cumsum(change) - 1
        within = np.arange(len(key)) - starts[runid]
        pos = seg_off[sc, sp] + within
        idx_all[r, pos] = si.astype(np.int16)
        # dstv relative to the slot's GROUP base pair: [0,256) for the base
        # pair, [256,512) for the next pair (straddle window)
        rel = sd + 256.0 * (sp - gpair_of_slot[pos])
        assert rel.min() >= 0 and rel.max() < 512
        dstv_all[r, pos] = rel

    idx_sb = np.stack([
        np.tile(idx_all[r].reshape(-1, 16).T, (8, 1)) for r in range(N_CORES)
    ])                                          # [8, 128, L/16]
    dstv_sb = np.stack([
        dstv_all[r].reshape(-1, 128).T for r in range(N_CORES)
    ]).astype(np.float16)                       # [8, 128, L/128] (ints exact)
    return dict(L=L, calls=calls, idx_sb=idx_sb, dstv_sb=dstv_sb)


def _prep(ei):
    src = ei[0].astype(np.int64)
    dst = ei[1].astype(np.int64)
    deg = np.bincount(dst, minlength=N).astype(np.float64) + 1.0
    dinv = (1.0 / np.sqrt(deg)).astype(np.float32)

    # ---------------- layer 1: all edges + self loops, dst-local layout
    # self loops form their own chunk (3) sourced from the per-core local
    # slice xsl, so their (chunk, pair) counts are identical on every core
    all_n = np.arange(N, dtype=np.int64)
    s1 = np.concatenate([src, all_n])
    d1 = np.concatenate([dst, all_n])
    core1 = d1 // NL
    du1 = d1 % NL
    E = len(src)
    c1_of = np.concatenate([src // CH1, np.full(N, 3, np.int64)])
    idxl1 = np.concatenate([src % CH1, all_n % NL])
    sch1 = _mk_schedule(core1, c1_of, du1 // 128,
                        (du1 % 256).astype(np.float32), idxl1,
                        T1, BLK1, NCH1)

    # ---------------- layer 2: host-dst edges + host self loops
    hm = (dst % GRAPH) < NH
    s2r, d2r = src[hm], dst[hm]
    hosts = all_n[(all_n % GRAPH) < NH]
    s2 = np.concatenate([s2r, hosts])
    d2 = np.concatenate([d2r, hosts])
    core2 = d2 // NL
    nloc = d2 % NL
    g2 = nloc // GRAPH
    h2 = nloc % GRAPH
    c2 = h2 * GPC + g2                          # compacted host slot
    # m2sf position: src s = r*NL + n -> AG tensor a = n//AGC,
    # row = (s//NL)*AGC + n%AGC
    sn = s2 % NL
    a2 = sn // AGC
    pos2 = (s2 // NL) * AGC + (sn % AGC)
    sch2 = _mk_schedule(core2, a2, c2 // 128,
                        (c2 % 256).astype(np.float32), pos2,
                        T2, BLK2, NCH2)

    # per-core dst-side dinv tables
    dinv_l = dinv.reshape(N_CORES, NL)
    dinv_fm = np.repeat(dinv_l[:, None, :], 128, axis=1)     # [8,128,NL]
    dinv_tiles = np.ascontiguousarray(
        dinv_l.reshape(N_CORES, T1, 128).transpose(0, 2, 1))  # [8,128,80]

    # head dst dinv: hzT[p, k, g] -> host h=2k+(p>=64), feat=p%64,
    # local node g*40+h
    dinv_hz = np.zeros((N_CORES, 128, 7, GPC), np.float32)
    for k in range(7):
        for half in range(2):
            h = 2 * k + half
            if h >= NH:
                continue
            nodes = np.arange(GPC) * GRAPH + h
            dinv_hz[:, 64 * half:64 * (half + 1), k, :] = \
                dinv_l[:, nodes][:, None, :]

    return dict(dinv=dinv, sch1=sch1, sch2=sch2, dinv_fm=dinv_fm,
                dinv_tiles=dinv_tiles, dinv_hz=dinv_hz)


# ---------------------------------------------------------------- builder

def _build(meta):
    sch1, sch2 = meta["sch1"], meta["sch2"]
    L1, L2 = sch1["L"], sch2["L"]
    nc = bacc.Bacc("TRN2", target_bir_lowering=False, debug=False,
                   num_devices=N_CORES, num_swdge_queues=4)
    d_xs = nc.dram_tensor("xs", [N, IN_DIM], BF16, kind="ExternalInput")
    d_xsl = nc.dram_tensor("xsl", [NL, IN_DIM], BF16, kind="ExternalInput")
    d_idx1 = nc.dram_tensor("idx1", [128, L1 // 16], I16, kind="ExternalInput")
    d_dstv1 = nc.dram_tensor("dstv1", [128, L1 // 128], F16,
                             kind="ExternalInput")
    d_idx2 = nc.dram_tensor("idx2", [128, L2 // 16], I16, kind="ExternalInput")
    d_dstv2 = nc.dram_tensor("dstv2", [128, L2 // 128], F16,
                             kind="ExternalInput")
    d_dinvfm = nc.dram_tensor("dinvfm", [128, NL], BF16, kind="ExternalInput")
    d_dinvt = nc.dram_tensor("dinvt", [128, T1], F32, kind="ExternalInput")
    d_dinvhz = nc.dram_tensor("dinvhz", [128, 7 * GPC], F32,
                              kind="ExternalInput")
    d_W1 = nc.dram_tensor("W1b", [IN_DIM, H1], BF16, kind="ExternalInput")
    d_b1 = nc.dram_tensor("b1p", [128, 2], F32, kind="ExternalInput")
    d_W2 = nc.dram_tensor("W2b", [128, 2 * H2], BF16, kind="ExternalInput")
    d_b2hz = nc.dram_tensor("b2hz", [128, 1], F32, kind="ExternalInput")
    d_Wout = nc.dram_tensor("Woutp", [128, 7 * ACT], F32, kind="ExternalInput")
    d_bout = nc.dram_tensor("bout", [1, ACT], F32, kind="ExternalInput")
    d_out = nc.dram_tensor("out", [GPC, ACT], F32, kind="ExternalOutput")

    qi = [0]   # global gather counter -> queue = qi % 4 (lane stays aligned)

    with tile.TileContext(nc) as tc, ExitStack() as top:
        perm = top.enter_context(tc.tile_pool(name="perm", bufs=1))
        wkc = top.enter_context(tc.tile_pool(name="wkc", bufs=2))
        dram = top.enter_context(tc.tile_pool(name="dram", bufs=1,
                                              space="DRAM"))

        # ---- persistent tiles (all loads via HWDGE to keep Pool clean)
        idx1t = perm.tile([128, L1 // 16], I16)
        nc.sync.dma_start(out=idx1t[:], in_=d_idx1[:])
        dstv1t = perm.tile([128, L1 // 128], F16)
        nc.sync.dma_start(out=dstv1t[:], in_=d_dstv1[:])
        idx2t = perm.tile([128, L2 // 16], I16)
        nc.sync.dma_start(out=idx2t[:], in_=d_idx2[:])
        dstv2t = perm.tile([128, L2 // 128], F16)
        nc.sync.dma_start(out=dstv2t[:], in_=d_dstv2[:])
        dinvfm = perm.tile([128, NL], BF16)
        nc.sync.dma_start(out=dinvfm[:], in_=d_dinvfm[:])
        dinvt = perm.tile([128, T1], F32)
        nc.sync.dma_start(out=dinvt[:], in_=d_dinvt[:])
        W1sb = perm.tile([128, H1], BF16)
        nc.sync.dma_start(out=W1sb[:], in_=d_W1[:])
        b1sb = perm.tile([128, 2], F32)
        nc.sync.dma_start(out=b1sb[:], in_=d_b1[:])
        W2sb = perm.tile([128, 2, H2], BF16)
        nc.sync.dma_start(out=W2sb[:].rearrange("p m f -> p (m f)"),
                          in_=d_W2[:])
        b2hz = perm.tile([128, 1], F32)
        nc.sync.dma_start(out=b2hz[:], in_=d_b2hz[:])
        WoutSB = perm.tile([128, 7, ACT], F32)
        nc.sync.dma_start(out=WoutSB[:].rearrange("p k a -> p (k a)"),
                          in_=d_Wout[:])
        boutrep = perm.tile([128, ACT], F32)
        nc.sync.dma_start(out=boutrep[:], in_=d_bout[:].to_broadcast((128, ACT)))
        dinvhz = perm.tile([128, 7, GPC], F32)
        nc.sync.dma_start(out=dinvhz[:].rearrange("p k g -> p (k g)"),
                          in_=d_dinvhz[:])

        zmm = perm.tile([128, 512], BF16)
        nc.gpsimd.memset(zmm[:], 0.0)
        ident = perm.tile([128, 128], F32)
        make_identity(nc, ident[:])
        iota_i = perm.tile([128, 256], I32)
        nc.gpsimd.iota(iota_i[:], pattern=[[1, 256]], base=0,
                       channel_multiplier=0)
        iota_bf = perm.tile([128, 256], F16)
        nc.vector.tensor_copy(out=iota_bf[:], in_=iota_i[:])
        iota_hi_i = perm.tile([128, 256], I32)
        nc.gpsimd.iota(iota_hi_i[:], pattern=[[1, 256]], base=256,
                       channel_multiplier=0)
        iota_hi = perm.tile([128, 256], F16)
        nc.vector.tensor_copy(out=iota_hi[:], in_=iota_hi_i[:])
        iota_f = perm.tile([128, 128], F32)
        nc.vector.tensor_copy(out=iota_f[:], in_=iota_i[:, 0:128])
        ioc = perm.tile([128, 1], I32)
        nc.gpsimd.iota(ioc[:], pattern=[[1, 1]], base=64, channel_multiplier=1)
        iocf = perm.tile([128, 1], F32)
        nc.vector.tensor_copy(out=iocf[:], in_=ioc[:])
        ident_hi = perm.tile([128, 128], F32)
        nc.vector.tensor_tensor(out=ident_hi[:],
                                in0=iocf[:].to_broadcast((128, 128)),
                                in1=iota_f[:], op=mybir.AluOpType.is_equal)

        m2sl = [dram.tile([AGC, H2], F32, name=f"m2sl{k}")
                for k in range(NCH2)]
        m2sf = [dram.tile([AGR, H2], F32, addr_space="Shared",
                          name=f"m2sf{k}") for k in range(NCH2)]
        agg2h = perm.tile([64, NHOST], F32)

        # split layer-1 calls by block for AllGather interleaving
        calls1_by_blk = [[] for _ in range(NB1)]
        for (c, s0, n_g, evs) in sch1["calls"]:
            b = evs[0][1] // (BLK1 // 2)
            calls1_by_blk[b].append((c, s0, n_g, evs))

        def gather(dst_tile, src_ap, idxt, s0, n_g, elem):
            nc.gpsimd.dma_gather(
                out_ap=dst_tile[:, 0:n_g, :],
                in_ap=src_ap,
                idxs_ap=idxt[:, s0 // 16: s0 // 16 + n_g * 8],
                num_idxs=n_g * 128, num_idxs_reg=n_g * 128,
                elem_size=elem, single_packet=False,
                queue_num=qi[0] % 4)
            qi[0] += 1

        def build_oh(wk, dstvt, s0, n_g):
            oh = wk.tile([128, CALL_G, 256], BF16, tag="oh", bufs=5)
            nc.vector.tensor_tensor(
                out=oh[:, 0:n_g, :],
                in0=dstvt[:, s0 // 128: s0 // 128 + n_g].unsqueeze(2)
                    .to_broadcast((128, n_g, 256)),
                in1=iota_bf[:].unsqueeze(1).to_broadcast((128, n_g, 256)),
                op=mybir.AluOpType.is_equal)
            return oh

        # =========================== Layer 1 ===========================
        with ExitStack() as ph1:
            wk1 = ph1.enter_context(tc.tile_pool(name="wk1", bufs=2))
            psA = ph1.enter_context(tc.tile_pool(name="psA", bufs=1,
                                                 space="PSUM"))
            psG = ph1.enter_context(tc.tile_pool(name="psG", bufs=2,
                                                 space="PSUM"))
            open_ps = {}   # super-tile (4 dst tiles per PSUM bank)

            def ps1_slice(t):
                sup = t // 4
                if sup not in open_ps:
                    ph = psA.tile([128, 512], F32, tag="agg",
                                  bufs=5, name=f"ps1s{sup}")
                    open_ps[sup] = ph
                    # start=True clears has_written for the WHOLE bank, so
                    # zero the full super once; all real events accumulate.
                    nc.tensor.matmul(out=ph[:], lhsT=zmm[:, 0:128],
                                     rhs=zmm[:], start=True, stop=False)
                return open_ps[sup][:, 128 * (t % 4):128 * (t % 4 + 1)]

            def ps1_pair(p):
                sup = p // 2
                if sup not in open_ps:
                    ps1_slice(4 * sup)      # opens + zeroes the super
                return open_ps[sup][:, 256 * (p % 2):256 * (p % 2 + 1)]

            def straddle_oh(wk, dstvt, s0, g):
                oh2 = wk.tile([128, 256], BF16, tag="oh2", bufs=3)
                col = s0 // 128 + g
                nc.vector.tensor_tensor(
                    out=oh2[:],
                    in0=dstvt[:, col:col + 1].to_broadcast((128, 256)),
                    in1=iota_hi[:], op=mybir.AluOpType.is_equal)
                return oh2[:]

            for b in range(NB1):
                for (c, s0, n_g, evs) in calls1_by_blk[b]:
                    if c < 3:
                        rows0, nrows = CH1_RANGES[c]
                        src_ap = d_xs[rows0:rows0 + nrows, :]
                    else:
                        src_ap = d_xsl[:]
                    gat = wk1.tile([128, CALL_G, IN_DIM], BF16, tag="gat",
                                   bufs=6)
                    gather(gat, src_ap, idx1t, s0, n_g, IN_DIM)
                    oh = build_oh(wk1, dstv1t, s0, n_g)
                    for (g, p, first, last, straddle) in evs:
                        rhs = (straddle_oh(wk1, dstv1t, s0, g) if straddle
                               else oh[:, g, :])
                        nc.tensor.matmul(
                            out=ps1_pair(p), lhsT=gat[:, g, :],
                            rhs=rhs, start=False, stop=last)
                # close tiles of this block: dinv scale + GEMM1 + GEMM2
                for t in range(b * BLK1, (b + 1) * BLK1):
                    ps = ps1_slice(t)
                    if t % 4 == 3:
                        del open_ps[t // 4]
                    aggt = wkc.tile([128, 128], BF16, tag="aggt", bufs=3)
                    nc.vector.tensor_tensor(
                        out=aggt[:], in0=ps,
                        in1=dinvfm[:, 128 * t:128 * (t + 1)],
                        op=mybir.AluOpType.mult)
                    phg = psG.tile([128, 320], F32, tag="h1g")
                    h1t = wkc.tile([128, 2, 128], BF16, tag="h1t", bufs=3)
                    for m in range(2):
                        nc.tensor.matmul(
                            out=phg[:, 128 * m:128 * (m + 1)],
                            lhsT=W1sb[:, 128 * m:128 * (m + 1)],
                            rhs=aggt[:], start=True, stop=True)
                        nc.scalar.activation(
                            out=h1t[:, m, :], in_=phg[:, 128 * m:128 * (m + 1)],
                            func=mybir.ActivationFunctionType.Relu,
                            bias=b1sb[:, m:m + 1], scale=1.0)
                    for m in range(2):
                        nc.tensor.matmul(out=phg[:, 256:320],
                                         lhsT=h1t[:, m, :],
                                         rhs=W2sb[:, m, :],
                                         start=(m == 0), stop=(m == 1))
                    if t % BLK1 == 0:
                        stgb = wkc.tile([128, BLK1, H2], F32, tag="stgb",
                                        bufs=2)
                    nc.scalar.activation(
                        out=stgb[:, t % BLK1, :], in_=phg[:, 256:320],
                        func=mybir.ActivationFunctionType.Copy,
                        scale=dinvt[:, t:t + 1])
                # one staging DMA per block; AllGather triggers must issue
                # from the Pool queue (walrus rejects other engines), so
                # AG_k goes after block k+1's gathers to avoid a Pool stall
                nc.sync.dma_start(
                    out=m2sl[b][:].rearrange("(t p) f -> p t f", p=128),
                    in_=stgb[:])
                ks = [b - 1] if b >= 1 else []
                for k in ks:
                    nc.gpsimd.collective_compute(
                        "AllGather", mybir.AluOpType.bypass,
                        replica_groups=[list(range(N_CORES))],
                        ins=[m2sl[k][:].opt()],
                        outs=[m2sf[k][:].opt()])

        # =========================== Layer 2 ===========================
        with ExitStack() as ph2:
            wk2 = ph2.enter_context(tc.tile_pool(name="wk2", bufs=2))
            psA2 = ph2.enter_context(tc.tile_pool(name="psA2", bufs=1,
                                                  space="PSUM"))
            open2 = {}

            def ps2_slice(t):
                sup = t // 4
                if sup not in open2:
                    ph = psA2.tile([64, 512], F32, tag="agg2",
                                   bufs=5, name=f"ps2s{sup}")
                    open2[sup] = ph
                    nc.tensor.matmul(out=ph[:], lhsT=zmm[:, 0:64],
                                     rhs=zmm[:], start=True, stop=False)
                return open2[sup][:, 128 * (t % 4):128 * (t % 4 + 1)]

            def ps2_pair(p):
                sup = p // 2
                if sup not in open2:
                    ps2_slice(4 * sup)
                return open2[sup][:, 256 * (p % 2):256 * (p % 2 + 1)]

            NP2 = T2 // 2
            for ci2, (c, s0, n_g, evs) in enumerate(sch2["calls"]):
                if ci2 == 3:
                    nc.gpsimd.collective_compute(
                        "AllGather", mybir.AluOpType.bypass,
                        replica_groups=[list(range(N_CORES))],
                        ins=[m2sl[NB1 - 1][:].opt()],
                        outs=[m2sf[NB1 - 1][:].opt()])
                gat2 = wk2.tile([128, CALL_G, H2], F32, tag="gat2", bufs=5)
                gather(gat2, m2sf[c][:], idx2t, s0, n_g, H2)
                g2b = wk2.tile([128, CALL_G, H2], BF16, tag="g2b", bufs=5)
                nc.scalar.activation(
                    out=g2b[:, 0:n_g, :], in_=gat2[:, 0:n_g, :],
                    func=mybir.ActivationFunctionType.Copy)
                oh = build_oh(wk2, dstv2t, s0, n_g)
                for (g, p, first, last, straddle) in evs:
                    rhs = (straddle_oh(wk2, dstv2t, s0, g) if straddle
                           else oh[:, g, :])
                    nc.tensor.matmul(
                        out=ps2_pair(p), lhsT=g2b[:, g, :],
                        rhs=rhs, start=False, stop=last)
                    if last:
                        nc.scalar.activation(
                            out=agg2h[:, 256 * p:256 * (p + 1)],
                            in_=ps2_pair(p),
                            func=mybir.ActivationFunctionType.Copy)
                        if p % 2 == 1 or p == NP2 - 1:
                            del open2[p // 2]

            # ===================== actor head ==========================
            with ExitStack() as ph4:
                mid4 = ph4.enter_context(tc.tile_pool(name="mid4", bufs=1))
                wk4 = ph4.enter_context(tc.tile_pool(name="wk4", bufs=2))
                psF = ph4.enter_context(tc.tile_pool(name="psF", bufs=2,
                                                     space="PSUM"))
                hzT = mid4.tile([128, 7, GPC], F32)
                h2r = agg2h[:].rearrange("p (q g) -> p q g", q=NH)
                for k in range(7):
                    hd = psF.tile([128, 416], F32, tag="hd", name=f"hzps{k}")
                    pk = hd[:, 0:GPC]
                    nc.tensor.matmul(out=pk, lhsT=ident[0:64, :],
                                     rhs=h2r[:, 2 * k, :],
                                     start=True, stop=(k == 6))
                    if k < 6:
                        nc.tensor.matmul(out=pk, lhsT=ident_hi[0:64, :],
                                         rhs=h2r[:, 2 * k + 1, :],
                                         start=False, stop=True)
                    nc.vector.tensor_tensor(out=hzT[:, k, :], in0=pk,
                                            in1=dinvhz[:, k, :],
                                            op=mybir.AluOpType.mult)
                nc.scalar.activation(out=hzT[:].rearrange("p k g -> p (k g)"),
                                     in_=hzT[:].rearrange("p k g -> p (k g)"),
                                     func=mybir.ActivationFunctionType.Relu,
                                     bias=b2hz[:, 0:1], scale=1.0)
                for m in range(GPC // 128):
                    hdf = psF.tile([128, 416], F32, tag="hd", name=f"finps{m}")
                    pf = hdf[:, 256:256 + ACT]
                    for k in range(6):
                        nc.tensor.matmul(
                            out=pf, lhsT=hzT[:, k, 128 * m:128 * (m + 1)],
                            rhs=WoutSB[:, k, :], start=(k == 0), stop=False)
                    nc.tensor.matmul(
                        out=pf, lhsT=hzT[0:64, 6, 128 * m:128 * (m + 1)],
                        rhs=WoutSB[0:64, 6, :], start=False, stop=True)
                    nc.vector.tensor_tensor(out=pf, in0=pf,
                                            in1=boutrep[:],
                                            op=mybir.AluOpType.add)
                    mx = wk4.tile([128, 1], F32, tag="mx")
                    nc.vector.tensor_reduce(out=mx[:], in_=pf,
                                            axis=mybir.AxisListType.X,
                                            op=mybir.AluOpType.max)
                    nmx = wk4.tile([128, 1], F32, tag="nmx")
                    nc.vector.tensor_scalar_mul(nmx[:], mx[:], -1.0)
                    esb = wk4.tile([128, ACT], F32, tag="esb")
                    nc.scalar.activation(out=esb[:], in_=pf,
                                         func=mybir.ActivationFunctionType.Exp,
                                         bias=nmx[:, 0:1], scale=1.0)
                    ssum = wk4.tile([128, 1], F32, tag="ssum")
                    nc.vector.tensor_reduce(out=ssum[:], in_=esb[:],
                                            axis=mybir.AxisListType.X,
                                            op=mybir.AluOpType.add)
                    rcp = wk4.tile([128, 1], F32, tag="rcp")
                    nc.vector.reciprocal(out=rcp[:], in_=ssum[:])
                    osb = wk4.tile([128, ACT], F32, tag="osb")
                    nc.vector.tensor_scalar_mul(osb[:], esb[:], rcp[:, 0:1])
                    nc.sync.dma_start(out=d_out[128 * m:128 * (m + 1), :],
                                      in_=osb[:])

    nc.compile()
    return nc


# ---------------------------------------------------------------- entry

_CACHE = {}


def _get(ei):
    key = hashlib.sha1(ei.tobytes()).hexdigest()
    if key not in _CACHE:
        meta = _prep(ei)
        nc = _build(meta)
        _CACHE[key] = (meta, nc)
    return _CACHE[key]


def _in_maps(meta, x, W1, b1, W2, b2, Wout, bout):
    dinv = meta["dinv"]
    xs = (x.astype(np.float32) * dinv[:, None]).astype(ml_dtypes.bfloat16)
    b1p = np.ascontiguousarray(
        np.asarray(b1, np.float32).reshape(2, 128).T)            # [128,2]
    W2p = np.ascontiguousarray(
        np.asarray(W2, np.float32).reshape(2, 128, H2).transpose(1, 0, 2)
        .reshape(128, 2 * H2)).astype(ml_dtypes.bfloat16)
    Woutp = np.zeros((128, 7, ACT), np.float32)
    for k in range(6):
        Woutp[:, k, :] = Wout[128 * k:128 * (k + 1), :]
    Woutp[0:64, 6, :] = Wout[768:832, :]
    b2t = np.tile(np.asarray(b2, np.float32).reshape(H2), 2).reshape(128, 1)
    maps = []
    for r in range(N_CORES):
        maps.append({
            "xs": xs,
            "xsl": np.ascontiguousarray(xs[r * NL:(r + 1) * NL]),
            "idx1": np.ascontiguousarray(meta["sch1"]["idx_sb"][r]),
            "dstv1": np.ascontiguousarray(meta["sch1"]["dstv_sb"][r]),
            "idx2": np.ascontiguousarray(meta["sch2"]["idx_sb"][r]),
            "dstv2": np.ascontiguousarray(meta["sch2"]["dstv_sb"][r]),
            "dinvfm": np.ascontiguousarray(meta["dinv_fm"][r])
                .astype(ml_dtypes.bfloat16),
            "dinvt": np.ascontiguousarray(meta["dinv_tiles"][r]),
            "dinvhz": np.ascontiguousarray(
                meta["dinv_hz"][r].reshape(128, 7 * GPC)),
            "W1b": np.ascontiguousarray(W1).astype(ml_dtypes.bfloat16),
            "b1p": b1p,
            "W2b": W2p,
            "b2hz": b2t,
            "Woutp": np.ascontiguousarray(Woutp.reshape(128, 7 * ACT)),
            "bout": np.ascontiguousarray(bout, np.float32).reshape(1, ACT),
        })
    return maps


def kernel(x, ei, W1, b1, W2, b2, Wout, bout, _trace=False):
    x = np.ascontiguousarray(x, np.float32)
    ei = np.ascontiguousarray(ei, np.int32)
    meta, nc = _get(ei)
    maps = _in_maps(meta, x, W1, b1, W2, b2, Wout, bout)
    res = bass_utils.run_bass_kernel_spmd(
        nc, maps, core_ids=list(range(N_CORES)), trace=_trace)
    out = np.concatenate([res.results[r]["out"] for r in range(N_CORES)],
                         axis=0).astype(np.float32)
    if _trace:
        return out, res.exec_time_ns
    return out


def install_profile_hook():
    import types
    sys.path.insert(0, "/root/.axon_site")
    import trn_agent_boot.trn_boot as _tb
    import antenv
    if "antenv.axon_hooks" not in sys.modules:
        _mod = types.ModuleType("antenv.axon_hooks")
        _h = [None]
        _mod.set_axon_ntff_profile_hook = lambda h: _h.__setitem__(0, h)
        _mod.get_axon_ntff_profile_hook = lambda: _h[0]
        sys.modules["antenv.axon_hooks"] = _mod
        antenv.axon_hooks = _mod
        _mod.set_axon_ntff_profile_hook(
            _tb._ntff_profile_via_ctypes("/opt/axon/libaxon_pjrt.so"))
